# revision 33
# baseline (speedup 1.0000x reference)
"""Arctic decoder layer (attention + residual MLP + top-2 MoE) on 8 TRN2 NeuronCores.

Strategy:
  - Data parallel over tokens for attention/norms/residual MLP (256 tokens/core,
    sliding-window attention needs only the previous 256-token chunk as halo).
  - Expert parallel for the MoE: every core receives the full (replicated) input,
    computes gating for all 2048 tokens, compacts the token indices routed to ITS
    expert (capacity 640), gathers them with indirect DMA, runs the expert FFN on
    the gathered tokens only, scales by the combine weights and scatters into a
    zeroed [2048, 1024] accumulator; one ReduceScatter(add) returns each core its
    own 256-token slice of the MoE output.
  - Activations live as [feature, token] (transposed) for matmuls; natural
    [token, feature] layout is used for RMS statistics, gating softmax/top-2 and
    the gather/scatter.  Matmuls run in bf16 (f32 PSUM accumulation); the gating
    logits use a bf16 split-float (hi+lo) product to keep top-2 selection exact.
"""
import os
import sys

for _p in ("/opt/trn_rl_repo", "/root/.axon_site/_ro/trn_rl_repo", "/root/.axon_site"):
    if os.path.isdir(_p) and _p not in sys.path:
        sys.path.append(_p)

import numpy as np

import concourse.bass as bass
import concourse.bacc as bacc
import concourse.mybir as mybir
import concourse.tile as tile
from concourse.bass_utils import run_bass_kernel_spmd
from concourse.masks import make_identity

F32 = mybir.dt.float32
BF16 = mybir.dt.bfloat16
I32 = mybir.dt.int32
AF = mybir.ActivationFunctionType
OP = mybir.AluOpType
AX = mybir.AxisListType

NCORES = 8
P = 128
B, S, H = 2, 1024, 1024
T = B * S                 # 2048 tokens
TT = T // P               # 16 token tiles
KH = H // P               # 8 hidden k-chunks
NH, NKV, HD = 16, 4, 64
F = 2816
FM = F // P               # 22
E = 8
CAP = 640                 # per-expert token capacity (actual max load is ~531)
G = CAP // P              # 5 slot batches
TS = T // NCORES          # 256 tokens per core
KV = 2 * TS               # 512 kv-window tokens per core
EPS = 1e-5
THETA = 10000.0
NEG = -1.0e5              # additive mask value (pre-exp)

_BUILD_CACHE = {}


def _build():
    if "nc" in _BUILD_CACHE:
        return _BUILD_CACHE["nc"]
    nc = bacc.Bacc("TRN2", target_bir_lowering=False, debug=False, num_devices=NCORES)

    dp = nc.declare_dram_parameter
    xT_kv = dp("xT_kv", [H, KV], F32, isOutput=False)
    xnat = dp("xnat", [T, H], BF16, isOutput=False)
    xT = dp("xT", [H, T], F32, isOutput=False)
    onehot = dp("onehot", [P, E], F32, isOutput=False)
    cos_q = dp("cos_q", [P, TS], F32, isOutput=False)
    sin_q = dp("sin_q", [P, TS], F32, isOutput=False)
    cos_k = dp("cos_k", [P, KV], F32, isOutput=False)
    sin_k = dp("sin_k", [P, KV], F32, isOutput=False)
    maskT = dp("maskT", [KV, TS], F32, isOutput=False)
    wq = dp("wq", [NH, P, KH * HD], BF16, isOutput=False)
    wk = dp("wk", [NKV, P, KH * HD], BF16, isOutput=False)
    wv = dp("wv", [2, P, H], BF16, isOutput=False)
    wo = dp("wo", [KH, HD, NH * P], BF16, isOutput=False)
    rw1 = dp("rw1", [KH, P, H], BF16, isOutput=False)
    rw3 = dp("rw3", [KH, P, H], BF16, isOutput=False)
    rw2 = dp("rw2", [KH, P, H], BF16, isOutput=False)
    ew1 = dp("ew1", [FM, P, H], BF16, isOutput=False)
    ew3 = dp("ew3", [FM, P, H], BF16, isOutput=False)
    ew2 = dp("ew2", [KH, P, F], BF16, isOutput=False)
    gatep = dp("gatep", [P, KH * E], F32, isOutput=False)
    out = dp("out", [H, TS], F32, isOutput=True)

    # internal DRAM (offset-0 targets for indirect DMA + collective bounces)
    xnorm_d = nc.dram_tensor("xnorm_d", [T, H], BF16)
    acc_d = nc.dram_tensor("acc_d", [T, H], BF16)
    rs_d = nc.dram_tensor("rs_d", [TS, H], BF16)

    with tile.TileContext(nc) as tc:
        with (
            tc.tile_pool(name="const", bufs=1) as cpool,
            tc.tile_pool(name="sb", bufs=2) as sb,
            tc.tile_pool(name="res", bufs=1) as res,
            tc.tile_pool(name="ps", bufs=2, space="PSUM") as ps,
            tc.tile_pool(name="ps1", bufs=1, space="PSUM") as ps1,
        ):
            # ---------------- constants ----------------
            idf = cpool.tile([P, P], F32)
            make_identity(nc, idf[:])
            idb = cpool.tile([P, P], BF16)
            make_identity(nc, idb[:])
            ones_b = cpool.tile([P, P], BF16)
            nc.vector.memset(ones_b[:], 1.0)
            # strict lower-triangular LT[k, m] = 1 if k < m (for exclusive cumsum)
            lt128 = cpool.tile([P, P], F32)
            nc.gpsimd.memset(lt128[:], 0.0)
            nc.gpsimd.affine_select(out=lt128[:], in_=lt128[:], pattern=[[-1, P]],
                                    compare_op=OP.is_ge, fill=1.0, base=0,
                                    channel_multiplier=1)
            lt16 = cpool.tile([TT, TT], F32)
            nc.gpsimd.memset(lt16[:], 0.0)
            nc.gpsimd.affine_select(out=lt16[:], in_=lt16[:], pattern=[[-1, TT]],
                                    compare_op=OP.is_ge, fill=1.0, base=0,
                                    channel_multiplier=1)
            # signed rotate-half permutation for RoPE: rot[m] = -q[m+32] | q[m-32]
            r64 = np.zeros((HD, HD), np.float32)
            for mm in range(32):
                r64[mm + 32, mm] = -1.0
                r64[mm, mm + 32] = 1.0
            r64_d = nc.inline_tensor(r64, name="r64_const")
            r64t = cpool.tile([HD, HD], F32)
            nc.sync.dma_start(out=r64t[:], in_=r64_d[:, :])
            epsb = cpool.tile([P, 1], F32)
            nc.vector.memset(epsb[:], EPS)
            zb = cpool.tile([P, H], BF16)
            nc.vector.memset(zb[:], 0.0)

            # ================= DP path (overlaps the collective) =============
            # D1: RMS over the 512-token kv window (transposed layout)
            ps_rms = ps.tile([P, KV], F32, tag="pA", space="PSUM")
            for k in range(KH):
                xk1 = sb.tile([P, KV], F32, tag="xkvS", name="xk1")
                nc.sync.dma_start(out=xk1[:], in_=xT_kv[k * P:(k + 1) * P, :])
                sqk = sb.tile([P, KV], BF16, tag="sqk")
                nc.scalar.activation(out=sqk[:], in_=xk1[:], func=AF.Square)
                nc.tensor.matmul(ps_rms[:], lhsT=ones_b[:], rhs=sqk[:],
                                 start=(k == 0), stop=(k == KH - 1))
            srk = sb.tile([P, KV], F32, tag="srk")
            nc.scalar.activation(out=srk[:], in_=ps_rms[:], func=AF.Sqrt,
                                 scale=1.0 / H, bias=epsb[:])
            rkv = sb.tile([P, KV], F32, tag="rkv", bufs=1)
            nc.vector.reciprocal(rkv[:], srk[:])
            xnkv = [res.tile([P, KV], BF16, tag=f"xnkv{k}", name=f"xnkv{k}") for k in range(KH)]
            for k in range(KH):
                xk2 = sb.tile([P, KV], F32, tag="xkvS", name="xk2")
                nc.sync.dma_start(out=xk2[:], in_=xT_kv[k * P:(k + 1) * P, :])
                nc.vector.tensor_mul(out=xnkv[k][:], in0=xk2[:], in1=rkv[:])

            # D2: q/k/v projections + RoPE + v transpose
            cq = cpool.tile([P, TS], F32)
            nc.sync.dma_start(out=cq[:], in_=cos_q[:, :])
            sq = cpool.tile([P, TS], F32)
            nc.sync.dma_start(out=sq[:], in_=sin_q[:, :])
            ck = cpool.tile([P, KV], F32)
            nc.sync.dma_start(out=ck[:], in_=cos_k[:, :])
            sk = cpool.tile([P, KV], F32)
            nc.sync.dma_start(out=sk[:], in_=sin_k[:, :])

            def rope(src_ps, cos_t, sin_t, w, dst, tagsfx):
                # src_ps: [HD, w] psum f32 (one head); dst: [HD, w] bf16 sbuf
                qf = sb.tile([HD, KV], F32, tag="ropeqf", name="ropeqf")
                nc.scalar.copy(qf[:, :w], src_ps[:, :w])
                rot = ps.tile([HD, KV], F32, tag="pC", space="PSUM", name="roperot")
                nc.tensor.matmul(rot[:, :w], lhsT=r64t[:], rhs=qf[:, :w],
                                 start=True, stop=True)
                t1 = sb.tile([HD, KV], F32, tag="ropet1", name="ropet1")
                nc.vector.tensor_mul(out=t1[:, :w], in0=qf[:, :w], in1=cos_t[0:HD, :w])
                nc.vector.tensor_mul(out=dst, in0=rot[:, :w], in1=sin_t[0:HD, :w])
                nc.vector.tensor_add(out=dst, in0=t1[:, :w], in1=dst)

            # per-head q (16 x [64, 256]) and per-kv-head k (4 x [64, 512])
            qrh = [res.tile([HD, TS], BF16, tag=f"qrh{h}", name=f"qrh{h}") for h in range(NH)]
            for h in range(NH):
                wqh = sb.tile([P, KH * HD], BF16, tag="wqh")
                nc.sync.dma_start(
                    out=wqh[:],
                    in_=wq[h, :, :])
                qp = ps.tile([HD, TS], F32, tag="pB", space="PSUM")
                for k in range(KH):
                    nc.tensor.matmul(qp[:], lhsT=wqh[:, k * HD:(k + 1) * HD],
                                     rhs=xnkv[k][:, TS:KV],
                                     start=(k == 0), stop=(k == KH - 1))
                rope(qp, cq, sq, TS, qrh[h][:], "q")
            krh = [res.tile([HD, KV], BF16, tag=f"krh{h}", name=f"krh{h}") for h in range(NKV)]
            vnat = [res.tile([P, NKV * HD], BF16, tag=f"vnat{c}", name=f"vnat{c}") for c in range(4)]
            for h in range(NKV):
                wkh = sb.tile([P, KH * HD], BF16, tag="wqh")
                nc.sync.dma_start(
                    out=wkh[:],
                    in_=wk[h, :, :])
                kp = ps.tile([HD, KV], F32, tag="pA", space="PSUM")
                for k in range(KH):
                    nc.tensor.matmul(kp[:], lhsT=wkh[:, k * HD:(k + 1) * HD],
                                     rhs=xnkv[k][:],
                                     start=(k == 0), stop=(k == KH - 1))
                rope(kp, ck, sk, KV, krh[h][:], "k")
            for m in range(2):
                wvm = sb.tile([P, H], BF16, tag="wqh")
                nc.sync.dma_start(
                    out=wvm[:],
                    in_=wv[m, :, :])
                vp = ps.tile([P, KV], F32, tag="pA", space="PSUM")
                for k in range(KH):
                    nc.tensor.matmul(vp[:], lhsT=wvm[:, k * P:(k + 1) * P],
                                     rhs=xnkv[k][:],
                                     start=(k == 0), stop=(k == KH - 1))
                vT = sb.tile([P, KV], BF16, tag="vT")
                nc.scalar.copy(vT[:], vp[:])
                for c in range(4):
                    ps_tp = ps.tile([P, P], BF16, tag="pB", space="PSUM")
                    nc.tensor.transpose(out=ps_tp[:], in_=vT[:, c * P:(c + 1) * P],
                                        identity=idb[:])
                    nc.scalar.copy(vnat[c][:, m * P:(m + 1) * P], ps_tp[:])


            # ---------------- M1: natural RMS over all tokens ----------------
            rinv_all = res.tile([P, TT], F32)
            for t in range(TT):
                xn = sb.tile([P, H], BF16, tag="xn")
                nc.gpsimd.dma_start(out=xn[:], in_=xnat[t * P:(t + 1) * P, :])
                sqs = sb.tile([P, H], BF16, tag="sqs")
                ssq = sb.tile([P, 1], F32, tag="ssq")
                nc.scalar.activation(out=sqs[:], in_=xn[:], func=AF.Square,
                                     accum_out=ssq[:])
                srt = sb.tile([P, 1], F32, tag="srt")
                nc.scalar.activation(out=srt[:], in_=ssq[:], func=AF.Sqrt,
                                     scale=1.0 / H, bias=epsb[:])
                nc.vector.reciprocal(rinv_all[:, t:t + 1], srt[:])
                xns = sb.tile([P, H], BF16, tag="xns")
                nc.scalar.activation(out=xns[:], in_=xn[:], func=AF.Copy,
                                     scale=rinv_all[:, t:t + 1])
                nc.gpsimd.dma_start(out=xnorm_d[t * P:(t + 1) * P, :], in_=xns[:])

            # ---------------- M2+M3+M4: gating ----------------
            gs = cpool.tile([P, KH * E], F32)
            nc.sync.dma_start(out=gs[:], in_=gatep[:, :])
            oh = cpool.tile([P, E], F32)
            nc.sync.dma_start(out=oh[:], in_=onehot[:, :])

            cw_all = res.tile([P, TT], F32)
            mask_all = res.tile([P, TT], F32)
            for n in range(T // 512):
                ps_lg = ps.tile([E, 512], F32, tag="pA", space="PSUM")
                for k in range(KH):
                    xs = sb.tile([P, 512], F32, tag="xsplit", bufs=5)
                    nc.sync.dma_start(
                        out=xs[:], in_=xT[k * P:(k + 1) * P, n * 512:(n + 1) * 512])
                    nc.tensor.matmul(
                        ps_lg[:], lhsT=gs[:, k * E:(k + 1) * E], rhs=xs[:],
                        start=(k == 0), stop=(k == KH - 1))
                lgT = sb.tile([E, 512], F32, tag="lgT")
                nc.scalar.copy(lgT[:], ps_lg[:])
                for j in range(4):
                    t = n * 4 + j
                    ps_tp = ps.tile([P, E], F32, tag="pB", space="PSUM")
                    nc.tensor.transpose(out=ps_tp[:], in_=lgT[:, j * P:(j + 1) * P],
                                        identity=idf[0:E, 0:E])
                    lg = sb.tile([P, E], F32, tag="lg")
                    nc.scalar.activation(out=lg[:], in_=ps_tp[:], func=AF.Copy,
                                         scale=rinv_all[:, t:t + 1])
                    # softmax + top2
                    ngm = sb.tile([P, 1], F32, tag="ngm")
                    nc.vector.tensor_reduce(out=ngm[:], in_=lg[:], axis=AX.X,
                                            op=OP.max, negate=True)
                    probs = sb.tile([P, E], F32, tag="probs")
                    nc.scalar.activation(out=probs[:], in_=lg[:], func=AF.Exp,
                                         bias=ngm[:])
                    top8 = sb.tile([P, E], F32, tag="top8")
                    nc.vector.max(out=top8[:], in_=probs[:])
                    den = sb.tile([P, 1], F32, tag="den")
                    nc.vector.tensor_add(out=den[:], in0=top8[:, 0:1], in1=top8[:, 1:2])
                    rden = sb.tile([P, 1], F32, tag="rden")
                    nc.vector.reciprocal(rden[:], den[:])
                    pex = sb.tile([P, E], F32, tag="pex")
                    nc.vector.tensor_mul(out=pex[:], in0=probs[:], in1=oh[:])
                    pe = sb.tile([P, 1], F32, tag="pe")
                    nc.vector.reduce_sum(out=pe[:], in_=pex[:], axis=AX.X)
                    nc.vector.tensor_tensor(out=mask_all[:, t:t + 1], in0=pe[:],
                                            in1=top8[:, 1:2], op=OP.is_ge)
                    cw0 = sb.tile([P, 1], F32, tag="cw0")
                    nc.vector.tensor_mul(out=cw0[:], in0=pe[:], in1=mask_all[:, t:t + 1])
                    nc.vector.tensor_mul(out=cw_all[:, t:t + 1], in0=cw0[:], in1=rden[:])

            # ---------------- M5: compaction ----------------
            ps_mt = ps.tile([TT, P], F32, tag="pB", space="PSUM")
            nc.tensor.transpose(out=ps_mt[:], in_=mask_all[:], identity=idf[:])
            mtp = sb.tile([TT, P], F32, tag="mtp")
            nc.scalar.copy(mtp[:], ps_mt[:])
            cs = sb.tile([TT, 1], F32, tag="cs")
            nc.vector.reduce_sum(out=cs[:], in_=mtp[:], axis=AX.X)
            ps_pos = ps.tile([P, TT], F32, tag="pA", space="PSUM")
            nc.tensor.matmul(ps_pos[:], lhsT=lt128[:], rhs=mask_all[:],
                             start=True, stop=False)
            nc.tensor.matmul(ps_pos[:], lhsT=cs[:].to_broadcast([TT, P]),
                             rhs=lt16[:], start=False, stop=True)
            slotf = sb.tile([P, TT], F32, tag="slotf")
            nc.vector.scalar_tensor_tensor(out=slotf[:], in0=ps_pos[:], scalar=4096.0,
                                           in1=mask_all[:], op0=OP.subtract, op1=OP.mult)
            nc.vector.tensor_scalar_add(slotf[:], slotf[:], 4096.0)
            # one-hot compaction: psc rows = [sum pid*oh, sum cw*oh, occ, sum t*oh]
            pid_i = sb.tile([P, 1], I32, tag="pid_i")
            nc.gpsimd.iota(pid_i[:], pattern=[[0, 1]], base=0, channel_multiplier=1)
            tv_i = sb.tile([P, TT], I32, tag="tv_i")
            nc.gpsimd.iota(tv_i[:], pattern=[[1, TT]], base=0, channel_multiplier=0)
            ic_scr = sb.tile([P, CAP], I32, tag="csb", bufs=1)
            nc.gpsimd.iota(ic_scr[:], pattern=[[1, CAP]], base=0, channel_multiplier=0)
            iotacols = cpool.tile([P, CAP], F32)
            nc.vector.tensor_copy(iotacols[:], ic_scr[:])
            lhs4 = cpool.tile([P, 4 * TT], BF16)
            lhs4v = lhs4.rearrange("p (t four) -> p t four", four=4)
            nc.vector.tensor_copy(lhs4v[:, :, 0], pid_i[:].to_broadcast([P, TT]))
            nc.vector.tensor_copy(lhs4v[:, :, 1], cw_all[:])
            nc.vector.memset(lhs4v[:, :, 2], 1.0)
            nc.vector.tensor_copy(lhs4v[:, :, 3], tv_i[:])
            psc_a = ps1.tile([4, 512], F32, tag="pd", space="PSUM")
            psc_b = ps1.tile([4, CAP - 512], F32, tag="po", space="PSUM")
            for t in range(TT):
                oh_t = sb.tile([P, CAP], BF16, tag="oh_t", bufs=2)
                nc.vector.tensor_scalar(out=oh_t[:], in0=iotacols[:],
                                        scalar1=slotf[:, t:t + 1], scalar2=None,
                                        op0=OP.is_equal)
                nc.tensor.matmul(psc_a[:], lhsT=lhs4[:, 4 * t:4 * t + 4],
                                 rhs=oh_t[:, 0:512],
                                 start=(t == 0), stop=(t == TT - 1))
                nc.tensor.matmul(psc_b[:], lhsT=lhs4[:, 4 * t:4 * t + 4],
                                 rhs=oh_t[:, 512:CAP],
                                 start=(t == 0), stop=(t == TT - 1))
            csb = sb.tile([4, CAP], F32, tag="csb", bufs=1)
            nc.scalar.copy(csb[:, 0:512], psc_a[:])
            nc.scalar.copy(csb[:, 512:CAP], psc_b[:])
            idx_i = res.tile([P, G], I32)
            cw_slots = res.tile([P, G], F32)
            for g in range(G):
                tpc = ps.tile([P, 4], F32, tag="pB", space="PSUM")
                nc.tensor.transpose(out=tpc[:], in_=csb[:, g * P:(g + 1) * P],
                                    identity=idf[0:4, 0:4])
                scr = sb.tile([P, 4], F32, tag="scr")
                nc.scalar.copy(scr[:], tpc[:])
                idxf = sb.tile([P, 1], F32, tag="idxf")
                nc.vector.scalar_tensor_tensor(out=idxf[:], in0=scr[:, 3:4],
                                               scalar=128.0, in1=scr[:, 0:1],
                                               op0=OP.mult, op1=OP.add)
                emp = sb.tile([P, 1], F32, tag="emp")
                nc.vector.tensor_scalar(out=emp[:], in0=scr[:, 2:3],
                                        scalar1=-2048.0, scalar2=2048.0,
                                        op0=OP.mult, op1=OP.add)
                nc.vector.tensor_add(out=idxf[:], in0=idxf[:], in1=emp[:])
                nc.vector.tensor_copy(idx_i[:, g:g + 1], idxf[:])
                nc.vector.tensor_copy(cw_slots[:, g:g + 1], scr[:, 1:2])

            # ---------------- M6: gather + transpose ----------------
            xgT = [res.tile([P, CAP], BF16, tag=f"xgT{k}", name=f"xgT{k}") for k in range(KH)]
            for g in range(G):
                gx = sb.tile([P, H], BF16, tag="gx")
                nc.vector.memset(gx[:], 0.0)
                nc.gpsimd.indirect_dma_start(
                    out=gx[:], out_offset=None, in_=xnorm_d[:, :],
                    in_offset=bass.IndirectOffsetOnAxis(ap=idx_i[:, g:g + 1], axis=0),
                    bounds_check=T - 1, oob_is_err=False)
                for k in range(KH):
                    ps_tp = ps.tile([P, P], BF16, tag="pB", space="PSUM")
                    nc.tensor.transpose(out=ps_tp[:], in_=gx[:, k * P:(k + 1) * P],
                                        identity=idb[:])
                    nc.scalar.copy(xgT[k][:, g * P:(g + 1) * P], ps_tp[:])

            # Residual-MLP weight preloads (issued interleaved with the FFN
            # streams so they complete before the ReduceScatter hogs the DMA
            # queues; recycled tags keep SBUF flat).
            _ptags = ["wqh", "wqh", "ropeqf",
                      "xsplit", "xsplit", "xsplit", "xsplit", "xsplit",
                      "xn", "xn", "sqs", "sqs", "xns", "xns", "gx", "gx",
                      "srk", "srk", "xkvS", "xkvS", "sqk", "sqk", "lgT", "lgT"]
            _psrc = [(rw1, m) for m in range(KH)] + [(rw3, m) for m in range(KH)] \
                    + [(rw2, m) for m in range(KH)]
            rwpre = []

            def emit_preload():
                i_ = len(rwpre)
                if i_ >= len(_psrc):
                    return
                wsrc, m = _psrc[i_]
                _tg = _ptags[i_]
                _bufs = {"xsplit": 5}.get(_tg, 2)
                tt_ = sb.tile([P, H], BF16, tag=_tg, name=f"rwpre{i_}", bufs=_bufs)
                nc.sync.dma_start(
                    out=tt_[:],
                    in_=wsrc[m, :, :])
                rwpre.append(tt_)

            # ---------------- M7: expert FFN on CAP slots ----------------
            NSC = ((0, 512), (512, CAP - 512))
            hT = [res.tile([P, CAP], BF16, tag=f"hT{m}", name=f"hT{m}") for m in range(FM)]
            for m in range(FM):
                w1m = sb.tile([P, H], BF16, tag="w1m", bufs=2)
                nc.sync.dma_start(
                    out=w1m[:],
                    in_=ew1[m, :, :])
                w3m = sb.tile([P, H], BF16, tag="w3m", bufs=2)
                nc.sync.dma_start(
                    out=w3m[:],
                    in_=ew3[m, :, :])
                p1a = ps.tile([P, 512], F32, tag="pA", space="PSUM", name="p1a")
                p1b = ps.tile([P, 128], F32, tag="pA", space="PSUM", name="p1b")
                p3a = ps.tile([P, 512], F32, tag="pB", space="PSUM", name="p3a")
                p3b = ps.tile([P, 128], F32, tag="pB", space="PSUM", name="p3b")
                for k in range(KH):
                    st, sp = k == 0, k == KH - 1
                    nc.tensor.matmul(p1a[:], lhsT=w1m[:, k * P:(k + 1) * P],
                                     rhs=xgT[k][:, 0:512], start=st, stop=sp)
                    nc.tensor.matmul(p1b[:], lhsT=w1m[:, k * P:(k + 1) * P],
                                     rhs=xgT[k][:, 512:CAP], start=st, stop=sp)
                for k in range(KH):
                    st, sp = k == 0, k == KH - 1
                    nc.tensor.matmul(p3a[:], lhsT=w3m[:, k * P:(k + 1) * P],
                                     rhs=xgT[k][:, 0:512], start=st, stop=sp)
                    nc.tensor.matmul(p3b[:], lhsT=w3m[:, k * P:(k + 1) * P],
                                     rhs=xgT[k][:, 512:CAP], start=st, stop=sp)
                emit_preload()
                for ns0, nsw, p1, p3 in ((0, 512, p1a, p3a), (512, CAP - 512, p1b, p3b)):
                    t1 = sb.tile([P, 512], BF16, tag="t1", name="t1")
                    nc.scalar.activation(out=t1[:, :nsw], in_=p1[:, :nsw], func=AF.Sigmoid)
                    tb = sb.tile([P, 512], BF16, tag="tb", name="tb")
                    nc.vector.tensor_tensor(out=tb[:, :nsw], in0=t1[:, :nsw],
                                            in1=p1[:, :nsw], op=OP.mult)
                    nc.vector.tensor_tensor(out=hT[m][:, ns0:ns0 + nsw],
                                            in0=tb[:, :nsw], in1=p3[:, :nsw], op=OP.mult)
            ynat = [res.tile([P, H], BF16, tag=f"ynat{g}", name=f"ynat{g}") for g in range(G)]
            for mh in range(KH):
                w2a = sb.tile([P, 11 * P], BF16, tag="w2m", bufs=2, name="w2a")
                nc.sync.dma_start(
                    out=w2a[:],
                    in_=ew2[mh, :, 0:11 * P])
                w2b = sb.tile([P, 11 * P], BF16, tag="w2m", bufs=2, name="w2b")
                nc.sync.dma_start(
                    out=w2b[:],
                    in_=ew2[mh, :, 11 * P:F])
                yT = sb.tile([P, CAP], BF16, tag="yT")
                pya = ps.tile([P, 512], F32, tag="pA", space="PSUM", name="pya")
                pyb = ps.tile([P, 128], F32, tag="pB", space="PSUM", name="pyb")
                for k in range(FM):
                    wsrc = w2a if k < 11 else w2b
                    lhs = wsrc[:, (k % 11) * P:(k % 11 + 1) * P]
                    st, sp = k == 0, k == FM - 1
                    nc.tensor.matmul(pya[:], lhsT=lhs, rhs=hT[k][:, 0:512],
                                     start=st, stop=sp)
                    nc.tensor.matmul(pyb[:], lhsT=lhs, rhs=hT[k][:, 512:CAP],
                                     start=st, stop=sp)
                nc.scalar.copy(yT[:, 0:512], pya[:])
                nc.scalar.copy(yT[:, 512:CAP], pyb[:])
                emit_preload()
                for g in range(G):
                    ps_tp = ps.tile([P, P], BF16, tag="pB", space="PSUM")
                    nc.tensor.transpose(out=ps_tp[:], in_=yT[:, g * P:(g + 1) * P],
                                        identity=idb[:])
                    nc.scalar.activation(out=ynat[g][:, mh * P:(mh + 1) * P],
                                         in_=ps_tp[:], func=AF.Copy,
                                         scale=cw_slots[:, g:g + 1])
            for t in range(TT):
                nc.sync.dma_start(out=acc_d[t * P:(t + 1) * P, :], in_=zb[:])
            for g in range(G):
                nc.gpsimd.indirect_dma_start(
                    out=acc_d[:, :],
                    out_offset=bass.IndirectOffsetOnAxis(ap=idx_i[:, g:g + 1], axis=0),
                    in_=ynat[g][:], in_offset=None,
                    bounds_check=T - 1, oob_is_err=False)

            rw1p, rw3p, rw2p = rwpre[0:KH], rwpre[KH:2 * KH], rwpre[2 * KH:3 * KH]

            # D3: attention per head (all tiles base-partition 0)
            mk = [cpool.tile([P, TS], F32, name=f"mk{c}") for c in range(4)]
            for c in range(4):
                nc.sync.dma_start(out=mk[c][:], in_=maskT[c * P:(c + 1) * P, :])
            attnh = [res.tile([HD, TS], BF16, tag=(f"attnh{h}" if h < 2 else f"qrh{h - 2}"), name=f"attnh{h}") for h in range(NH)]
            for h in range(NH):
                kvh = h // 4
                pd = ps1.tile([P, TS], F32, tag="pd", space="PSUM")
                po = ps1.tile([HD, TS], F32, tag="po", space="PSUM")
                for c in range(4):
                    ps_s = ps.tile([P, TS], F32, tag="pC", space="PSUM")
                    nc.tensor.matmul(ps_s[:],
                                     lhsT=krh[kvh][:, c * P:(c + 1) * P],
                                     rhs=qrh[h][:], start=True, stop=True)
                    sm = sb.tile([P, TS], F32, tag="sm")
                    nc.vector.tensor_add(out=sm[:], in0=ps_s[:], in1=mk[c][:])
                    pT = sb.tile([P, TS], BF16, tag="pT", bufs=3)
                    nc.scalar.activation(out=pT[:], in_=sm[:], func=AF.Exp, scale=0.125)
                    nc.tensor.matmul(pd[:], lhsT=ones_b[:], rhs=pT[:],
                                     start=(c == 0), stop=(c == 3))
                    nc.tensor.matmul(po[:], lhsT=vnat[c][:, kvh * HD:(kvh + 1) * HD],
                                     rhs=pT[:], start=(c == 0), stop=(c == 3))
                rd = sb.tile([HD, TS], F32, tag="rd")
                nc.vector.reciprocal(rd[:], pd[0:HD, :])
                nc.vector.tensor_tensor(out=attnh[h][:], in0=po[:],
                                        in1=rd[:], op=OP.mult)

            # D4: output projection (contraction in 16 chunks of 64) + residual
            RAT = [res.tile([P, TS], F32, tag=f"xgT{m}", name=f"RAT{m}") for m in range(KH)]
            for m in range(KH):
                woa = sb.tile([HD, 8 * P], BF16, tag="wom", bufs=2, name="woa")
                nc.sync.dma_start(
                    out=woa[:],
                    in_=wo[m, :, 0:8 * P])
                wob = sb.tile([HD, 8 * P], BF16, tag="wom", bufs=2, name="wob")
                nc.sync.dma_start(
                    out=wob[:],
                    in_=wo[m, :, 8 * P:NH * P])
                op_ps = ps.tile([P, TS], F32, tag="pB", space="PSUM")
                for k in range(NH):
                    wsrc = woa if k < 8 else wob
                    nc.tensor.matmul(op_ps[:], lhsT=wsrc[:, (k % 8) * P:(k % 8 + 1) * P],
                                     rhs=attnh[k][:], start=(k == 0), stop=(k == NH - 1))
                xres = sb.tile([P, TS], F32, tag="xres", name="xres")
                nc.sync.dma_start(out=xres[:], in_=xT_kv[m * P:(m + 1) * P, TS:KV])
                nc.vector.tensor_add(out=RAT[m][:], in0=op_ps[:], in1=xres[:])

            # D5: residual MLP
            ps_rm = ps.tile([P, TS], F32, tag="pA", space="PSUM")
            for m in range(KH):
                sqm = sb.tile([P, TS], BF16, tag="sqm")
                nc.scalar.activation(out=sqm[:], in_=RAT[m][:], func=AF.Square)
                nc.tensor.matmul(ps_rm[:], lhsT=ones_b[:], rhs=sqm[:],
                                 start=(m == 0), stop=(m == KH - 1))
            srm = sb.tile([P, TS], F32, tag="srm")
            nc.scalar.activation(out=srm[:], in_=ps_rm[:], func=AF.Sqrt,
                                 scale=1.0 / H, bias=epsb[:])
            rrm = sb.tile([P, TS], F32, tag="rrm", bufs=1)
            nc.vector.reciprocal(rrm[:], srm[:])
            xmT = [res.tile([P, TS], BF16, tag=f"hT{16 + m}" if m < 6 else f"ynat{m - 6}", name=f"xmT{m}") for m in range(KH)]
            for m in range(KH):
                nc.vector.tensor_mul(out=xmT[m][:], in0=RAT[m][:], in1=rrm[:])
            hm = [res.tile([P, TS], BF16, tag=f"hT{8 + m}", name=f"hm{m}") for m in range(KH)]
            for m in range(KH):
                p1 = ps.tile([P, TS], F32, tag="pB", space="PSUM")
                for k in range(KH):
                    nc.tensor.matmul(p1[:], lhsT=rw1p[m][:, k * P:(k + 1) * P],
                                     rhs=xmT[k][:], start=(k == 0), stop=(k == KH - 1))
                p3 = ps.tile([P, TS], F32, tag="pC", space="PSUM")
                for k in range(KH):
                    nc.tensor.matmul(p3[:], lhsT=rw3p[m][:, k * P:(k + 1) * P],
                                     rhs=xmT[k][:], start=(k == 0), stop=(k == KH - 1))
                t1 = sb.tile([P, TS], BF16, tag="t1d")
                nc.scalar.activation(out=t1[:], in_=p1[:], func=AF.Sigmoid)
                tb = sb.tile([P, TS], BF16, tag="tbd")
                nc.vector.tensor_tensor(out=tb[:], in0=t1[:], in1=p1[:], op=OP.mult)
                nc.vector.tensor_tensor(out=hm[m][:], in0=tb[:], in1=p3[:], op=OP.mult)

            # D6a: rw2 + residual accumulated in place into RAT (pre-collective)
            for m in range(KH):
                p2 = ps.tile([P, TS], F32, tag="pB", space="PSUM")
                for k in range(KH):
                    nc.tensor.matmul(p2[:], lhsT=rw2p[m][:, k * P:(k + 1) * P],
                                     rhs=hm[k][:], start=(k == 0), stop=(k == KH - 1))
                nc.vector.tensor_add(out=RAT[m][:], in0=p2[:], in1=RAT[m][:])
            # ---------------- M8: ReduceScatter ----------------
            nc.gpsimd.collective_compute(
                "ReduceScatter", OP.add, replica_groups=[list(range(NCORES))],
                ins=[acc_d.ap().opt()], outs=[rs_d.ap().opt()])

            # D6b: MoE slice transpose + final sum -> output
            moeT = [res.tile([P, TS], F32, tag=f"hT{k}", name=f"moeT{k}") for k in range(KH)]
            for pt in range(2):
                rsb = sb.tile([P, H], BF16, tag="rsb")
                nc.gpsimd.dma_start(out=rsb[:], in_=rs_d[pt * P:(pt + 1) * P, :])
                for k in range(KH):
                    ps_tp = ps.tile([P, P], BF16, tag="pB", space="PSUM")
                    nc.tensor.transpose(out=ps_tp[:], in_=rsb[:, k * P:(k + 1) * P],
                                        identity=idb[:])
                    nc.scalar.copy(moeT[k][:, pt * P:(pt + 1) * P], ps_tp[:])
            for m in range(KH):
                ot = sb.tile([P, TS], F32, tag="ot")
                nc.vector.tensor_add(out=ot[:], in0=RAT[m][:], in1=moeT[m][:])
                nc.gpsimd.dma_start(out=out[m * P:(m + 1) * P, :], in_=ot[:])

    nc.finalize()
    _BUILD_CACHE["nc"] = nc
    return nc


def _host_prep(inputs):
    f32 = np.float32
    x = np.asarray(inputs["hidden_states"], f32).reshape(T, H)
    ln1 = np.asarray(inputs["ln1_w"], f32)
    res_ln = np.asarray(inputs["res_ln_w"], f32)
    post_ln = np.asarray(inputs["post_ln_w"], f32)

    import ml_dtypes
    bf16 = ml_dtypes.bfloat16

    def b(a):
        return np.ascontiguousarray(np.asarray(a, f32)).astype(bf16)

    def mmaj(w, pp, mm):
        # [K, M] -> [M//mm, pp, (K//pp)*mm] with w[k, m] at [m//mm, k%pp, (k//pp)*mm + m%mm]
        K, M = w.shape
        return np.ascontiguousarray(
            w.reshape(K // pp, pp, M // mm, mm).transpose(2, 1, 0, 3).reshape(M // mm, pp, (K // pp) * mm))

    wq = mmaj(b(ln1[:, None] * np.asarray(inputs["q_w"], f32)), 128, 64)
    wk = mmaj(b(ln1[:, None] * np.asarray(inputs["k_w"], f32)), 128, 64)
    wv = mmaj(b(ln1[:, None] * np.asarray(inputs["v_w"], f32)), 128, 128)
    wo = mmaj(b(inputs["o_w"]), 64, 128)
    rw1 = mmaj(b(res_ln[:, None] * np.asarray(inputs["rw1"], f32)), 128, 128)
    rw3 = mmaj(b(res_ln[:, None] * np.asarray(inputs["rw3"], f32)), 128, 128)
    rw2 = mmaj(b(inputs["rw2"]), 128, 128)
    gate = np.ascontiguousarray(post_ln[:, None] * np.asarray(inputs["gate_w"], f32))
    gatep = np.ascontiguousarray(gate.reshape(8, 128, 8).transpose(1, 0, 2).reshape(128, 64))
    xT = np.ascontiguousarray(x.T)                       # [H, T]

    e_w1 = np.asarray(inputs["e_w1"], f32)
    e_w3 = np.asarray(inputs["e_w3"], f32)
    e_w2 = np.asarray(inputs["e_w2"], f32)

    # RoPE tables: cos64[d, pos] with d in [0,64), duplicated inv-freq halves
    pos = np.arange(S, dtype=f32)
    inv = 1.0 / (THETA ** (np.arange(0, HD, 2, dtype=f32) / HD))   # [32]
    ang = inv[:, None] * pos[None, :]                               # [32, S]
    cos64 = np.concatenate([np.cos(ang)] * 2, 0)                    # [64, S]
    sin64 = np.concatenate([np.sin(ang)] * 2, 0)

    in_maps = []
    for core in range(NCORES):
        bi, c = divmod(core, 4)
        lo = bi * S + c * TS
        # kv window: previous chunk + own chunk (zeros for c == 0)
        xkv = np.zeros((H, KV), f32)
        if c > 0:
            xkv[:, :TS] = xT[:, lo - TS:lo]
        xkv[:, TS:] = xT[:, lo:lo + TS]
        # mask: valid iff ql < kl <= ql + TS (and kl >= TS when c == 0)
        ql = np.arange(TS)[None, :]
        kl = np.arange(KV)[:, None]
        valid = (kl > ql) & (kl <= ql + TS)
        if c == 0:
            valid &= kl >= TS
        maskT = np.where(valid, 0.0, NEG).astype(f32)
        # RoPE positions (within-sequence)
        pq = c * TS + np.arange(TS)
        pk = np.clip((c - 1) * TS + np.arange(KV), 0, S - 1)
        cq = np.tile(cos64[:, pq], (2, 1)).astype(f32)
        sqv = np.tile(sin64[:, pq], (2, 1)).astype(f32)
        ckv = np.tile(cos64[:, pk], (2, 1)).astype(f32)
        skv = np.tile(sin64[:, pk], (2, 1)).astype(f32)
        oh = np.zeros((P, E), f32)
        oh[:, core] = 1.0
        in_maps.append(dict(
            xT_kv=xkv, xnat=x.astype(bf16), xT=xT, gatep=gatep, onehot=oh,
            cos_q=cq, sin_q=sqv, cos_k=ckv, sin_k=skv, maskT=maskT,
            wq=wq, wk=wk, wv=wv, wo=wo, rw1=rw1, rw3=rw3, rw2=rw2,
            ew1=mmaj(b(post_ln[:, None] * e_w1[core]), 128, 128),
            ew3=mmaj(b(post_ln[:, None] * e_w3[core]), 128, 128),
            ew2=mmaj(b(e_w2[core]), 128, 128),
        ))
    return in_maps


def kernel(**inputs) -> np.ndarray:
    nc = _build()
    in_maps = _host_prep(inputs)
    res = run_bass_kernel_spmd(nc, in_maps, core_ids=list(range(NCORES)))
    outs = [np.asarray(res.results[i]["out"], np.float32).T for i in range(NCORES)]
    full = np.concatenate(outs, 0)          # [T, H] in core order == token order
    return full.reshape(B, S, H)


# revision 34
# speedup vs baseline: 1.0579x; 1.0579x over previous
"""Arctic decoder layer (attention + residual MLP + top-2 MoE) on 8 TRN2 NeuronCores.

Strategy:
  - Data parallel over tokens for attention/norms/residual MLP (256 tokens/core,
    sliding-window attention needs only the previous 256-token chunk as halo).
  - Expert parallel for the MoE: every core receives the full (replicated) input,
    computes gating for all 2048 tokens, compacts the token indices routed to ITS
    expert (capacity 640), gathers them with indirect DMA, runs the expert FFN on
    the gathered tokens only, scales by the combine weights and scatters into a
    zeroed [2048, 1024] accumulator; one ReduceScatter(add) returns each core its
    own 256-token slice of the MoE output.
  - Activations live as [feature, token] (transposed) for matmuls; natural
    [token, feature] layout is used for RMS statistics, gating softmax/top-2 and
    the gather/scatter.  Matmuls run in bf16 (f32 PSUM accumulation); the gating
    logits use a bf16 split-float (hi+lo) product to keep top-2 selection exact.
"""
import os
import sys

for _p in ("/opt/trn_rl_repo", "/root/.axon_site/_ro/trn_rl_repo", "/root/.axon_site"):
    if os.path.isdir(_p) and _p not in sys.path:
        sys.path.append(_p)

import numpy as np

import concourse.bass as bass
import concourse.bacc as bacc
import concourse.mybir as mybir
import concourse.tile as tile
from concourse.bass_utils import run_bass_kernel_spmd
from concourse.masks import make_identity

F32 = mybir.dt.float32
BF16 = mybir.dt.bfloat16
I32 = mybir.dt.int32
AF = mybir.ActivationFunctionType
OP = mybir.AluOpType
AX = mybir.AxisListType

NCORES = 8
P = 128
B, S, H = 2, 1024, 1024
T = B * S                 # 2048 tokens
TT = T // P               # 16 token tiles
KH = H // P               # 8 hidden k-chunks
NH, NKV, HD = 16, 4, 64
F = 2816
FM = F // P               # 22
E = 8
CAP = 640                 # per-expert token capacity (actual max load is ~531)
G = CAP // P              # 5 slot batches
TS = T // NCORES          # 256 tokens per core
KV = 2 * TS               # 512 kv-window tokens per core
EPS = 1e-5
THETA = 10000.0
NEG = -1.0e5              # additive mask value (pre-exp)

_BUILD_CACHE = {}


def _build():
    if "nc" in _BUILD_CACHE:
        return _BUILD_CACHE["nc"]
    nc = bacc.Bacc("TRN2", target_bir_lowering=False, debug=False, num_devices=NCORES)

    dp = nc.declare_dram_parameter
    xT_kv = dp("xT_kv", [H, KV], F32, isOutput=False)
    xnat = dp("xnat", [T, H], BF16, isOutput=False)
    xT = dp("xT", [H, T], F32, isOutput=False)
    onehot = dp("onehot", [P, E], F32, isOutput=False)
    cos_q = dp("cos_q", [P, TS], F32, isOutput=False)
    sin_q = dp("sin_q", [P, TS], F32, isOutput=False)
    cos_k = dp("cos_k", [P, KV], F32, isOutput=False)
    sin_k = dp("sin_k", [P, KV], F32, isOutput=False)
    maskT = dp("maskT", [KV, TS], F32, isOutput=False)
    wq = dp("wq", [NH, P, KH * HD], BF16, isOutput=False)
    wk = dp("wk", [NKV, P, KH * HD], BF16, isOutput=False)
    wv = dp("wv", [2, P, H], BF16, isOutput=False)
    wo = dp("wo", [KH, HD, NH * P], BF16, isOutput=False)
    rw1 = dp("rw1", [KH, P, H], BF16, isOutput=False)
    rw3 = dp("rw3", [KH, P, H], BF16, isOutput=False)
    rw2 = dp("rw2", [KH, P, H], BF16, isOutput=False)
    ew1 = dp("ew1", [FM, P, H], BF16, isOutput=False)
    ew3 = dp("ew3", [FM, P, H], BF16, isOutput=False)
    ew2 = dp("ew2", [KH, P, F], BF16, isOutput=False)
    gatep = dp("gatep", [P, KH * E], F32, isOutput=False)
    out = dp("out", [H, TS], F32, isOutput=True)

    # internal DRAM (offset-0 targets for indirect DMA + collective bounces)
    xnorm_d = nc.dram_tensor("xnorm_d", [T, H], BF16)
    acc_d = nc.dram_tensor("acc_d", [T, H], BF16)
    rs_d = nc.dram_tensor("rs_d", [TS, H], BF16)

    with tile.TileContext(nc) as tc:
        with (
            tc.tile_pool(name="const", bufs=1) as cpool,
            tc.tile_pool(name="sb", bufs=2) as sb,
            tc.tile_pool(name="res", bufs=1) as res,
            tc.tile_pool(name="ps", bufs=2, space="PSUM") as ps,
            tc.tile_pool(name="ps1", bufs=1, space="PSUM") as ps1,
        ):
            # ---------------- constants ----------------
            idf = cpool.tile([P, P], F32)
            make_identity(nc, idf[:])
            idb = cpool.tile([P, P], BF16)
            make_identity(nc, idb[:])
            ones_b = cpool.tile([P, P], BF16)
            nc.vector.memset(ones_b[:], 1.0)
            # strict lower-triangular LT[k, m] = 1 if k < m (for exclusive cumsum)
            lt128 = cpool.tile([P, P], F32)
            nc.gpsimd.memset(lt128[:], 0.0)
            nc.gpsimd.affine_select(out=lt128[:], in_=lt128[:], pattern=[[-1, P]],
                                    compare_op=OP.is_ge, fill=1.0, base=0,
                                    channel_multiplier=1)
            lt16 = cpool.tile([TT, TT], F32)
            nc.gpsimd.memset(lt16[:], 0.0)
            nc.gpsimd.affine_select(out=lt16[:], in_=lt16[:], pattern=[[-1, TT]],
                                    compare_op=OP.is_ge, fill=1.0, base=0,
                                    channel_multiplier=1)
            # signed rotate-half permutation for RoPE: rot[m] = -q[m+32] | q[m-32]
            r64 = np.zeros((HD, HD), np.float32)
            for mm in range(32):
                r64[mm + 32, mm] = -1.0
                r64[mm, mm + 32] = 1.0
            r64_d = nc.inline_tensor(r64, name="r64_const")
            r64t = cpool.tile([HD, HD], F32)
            nc.sync.dma_start(out=r64t[:], in_=r64_d[:, :])
            epsb = cpool.tile([P, 1], F32)
            nc.vector.memset(epsb[:], EPS)
            zb = cpool.tile([P, H], BF16)
            nc.vector.memset(zb[:], 0.0)

            # ================= DP path (overlaps the collective) =============
            # D1: RMS over the 512-token kv window (transposed layout)
            ps_rms = ps.tile([P, KV], F32, tag="pA", space="PSUM")
            for k in range(KH):
                xk1 = sb.tile([P, KV], F32, tag="xkvS", name="xk1")
                nc.sync.dma_start(out=xk1[:], in_=xT_kv[k * P:(k + 1) * P, :])
                sqk = sb.tile([P, KV], BF16, tag="sqk")
                nc.scalar.activation(out=sqk[:], in_=xk1[:], func=AF.Square)
                nc.tensor.matmul(ps_rms[:], lhsT=ones_b[:], rhs=sqk[:],
                                 start=(k == 0), stop=(k == KH - 1))
            srk = sb.tile([P, KV], F32, tag="srk")
            nc.scalar.activation(out=srk[:], in_=ps_rms[:], func=AF.Sqrt,
                                 scale=1.0 / H, bias=epsb[:])
            rkv = sb.tile([P, KV], F32, tag="rkv", bufs=1)
            nc.vector.reciprocal(rkv[:], srk[:])
            xnkv = [res.tile([P, KV], BF16, tag=f"xnkv{k}", name=f"xnkv{k}") for k in range(KH)]
            for k in range(KH):
                xk2 = sb.tile([P, KV], F32, tag="xkvS", name="xk2")
                nc.sync.dma_start(out=xk2[:], in_=xT_kv[k * P:(k + 1) * P, :])
                nc.vector.tensor_mul(out=xnkv[k][:], in0=xk2[:], in1=rkv[:])

            # D2: q/k/v projections + RoPE + v transpose
            cq = cpool.tile([P, TS], F32)
            nc.sync.dma_start(out=cq[:], in_=cos_q[:, :])
            sq = cpool.tile([P, TS], F32)
            nc.sync.dma_start(out=sq[:], in_=sin_q[:, :])
            ck = cpool.tile([P, KV], F32)
            nc.sync.dma_start(out=ck[:], in_=cos_k[:, :])
            sk = cpool.tile([P, KV], F32)
            nc.sync.dma_start(out=sk[:], in_=sin_k[:, :])

            def rope(src_ps, cos_t, sin_t, w, dst, tagsfx):
                # src_ps: [HD, w] psum f32 (one head); dst: [HD, w] bf16 sbuf
                qf = sb.tile([HD, KV], F32, tag="ropeqf", name="ropeqf")
                nc.scalar.copy(qf[:, :w], src_ps[:, :w])
                rot = ps.tile([HD, KV], F32, tag="pC", space="PSUM", name="roperot")
                nc.tensor.matmul(rot[:, :w], lhsT=r64t[:], rhs=qf[:, :w],
                                 start=True, stop=True)
                t1 = sb.tile([HD, KV], F32, tag="ropet1", name="ropet1")
                nc.vector.tensor_mul(out=t1[:, :w], in0=qf[:, :w], in1=cos_t[0:HD, :w])
                nc.vector.tensor_mul(out=dst, in0=rot[:, :w], in1=sin_t[0:HD, :w])
                nc.vector.tensor_add(out=dst, in0=t1[:, :w], in1=dst)

            # per-head q (16 x [64, 256]) and per-kv-head k (4 x [64, 512])
            qrh = [res.tile([HD, TS], BF16, tag=f"qrh{h}", name=f"qrh{h}") for h in range(NH)]
            for h in range(NH):
                wqh = sb.tile([P, KH * HD], BF16, tag="wqh")
                nc.sync.dma_start(
                    out=wqh[:],
                    in_=wq[h, :, :])
                qp = ps.tile([HD, TS], F32, tag="pB", space="PSUM")
                for k in range(KH):
                    nc.tensor.matmul(qp[:], lhsT=wqh[:, k * HD:(k + 1) * HD],
                                     rhs=xnkv[k][:, TS:KV],
                                     start=(k == 0), stop=(k == KH - 1))
                rope(qp, cq, sq, TS, qrh[h][:], "q")
            krh = [res.tile([HD, KV], BF16, tag=f"krh{h}", name=f"krh{h}") for h in range(NKV)]
            vnat = [res.tile([P, NKV * HD], BF16, tag=f"vnat{c}", name=f"vnat{c}") for c in range(4)]
            for h in range(NKV):
                wkh = sb.tile([P, KH * HD], BF16, tag="wqh")
                nc.sync.dma_start(
                    out=wkh[:],
                    in_=wk[h, :, :])
                kp = ps.tile([HD, KV], F32, tag="pA", space="PSUM")
                for k in range(KH):
                    nc.tensor.matmul(kp[:], lhsT=wkh[:, k * HD:(k + 1) * HD],
                                     rhs=xnkv[k][:],
                                     start=(k == 0), stop=(k == KH - 1))
                rope(kp, ck, sk, KV, krh[h][:], "k")
            for m in range(2):
                wvm = sb.tile([P, H], BF16, tag="wqh")
                nc.sync.dma_start(
                    out=wvm[:],
                    in_=wv[m, :, :])
                vp = ps.tile([P, KV], F32, tag="pA", space="PSUM")
                for k in range(KH):
                    nc.tensor.matmul(vp[:], lhsT=wvm[:, k * P:(k + 1) * P],
                                     rhs=xnkv[k][:],
                                     start=(k == 0), stop=(k == KH - 1))
                vT = sb.tile([P, KV], BF16, tag="vT")
                nc.scalar.copy(vT[:], vp[:])
                for c in range(4):
                    ps_tp = ps.tile([P, P], BF16, tag="pB", space="PSUM")
                    nc.tensor.transpose(out=ps_tp[:], in_=vT[:, c * P:(c + 1) * P],
                                        identity=idb[:])
                    nc.scalar.copy(vnat[c][:, m * P:(m + 1) * P], ps_tp[:])


            # ---------------- M1: natural RMS over all tokens ----------------
            rinv_all = res.tile([P, TT], F32)
            for t in range(TT):
                xn = sb.tile([P, H], BF16, tag="xn")
                nc.sync.dma_start(out=xn[:], in_=xnat[t * P:(t + 1) * P, :])
                sqs = sb.tile([P, H], BF16, tag="sqs")
                ssq = sb.tile([P, 1], F32, tag="ssq")
                nc.scalar.activation(out=sqs[:], in_=xn[:], func=AF.Square,
                                     accum_out=ssq[:])
                srt = sb.tile([P, 1], F32, tag="srt")
                nc.scalar.activation(out=srt[:], in_=ssq[:], func=AF.Sqrt,
                                     scale=1.0 / H, bias=epsb[:])
                nc.vector.reciprocal(rinv_all[:, t:t + 1], srt[:])
                xns = sb.tile([P, H], BF16, tag="xns")
                nc.scalar.activation(out=xns[:], in_=xn[:], func=AF.Copy,
                                     scale=rinv_all[:, t:t + 1])
                nc.sync.dma_start(out=xnorm_d[t * P:(t + 1) * P, :], in_=xns[:])

            # ---------------- M2+M3+M4: gating ----------------
            gs = cpool.tile([P, KH * E], F32)
            nc.sync.dma_start(out=gs[:], in_=gatep[:, :])
            oh = cpool.tile([P, E], F32)
            nc.sync.dma_start(out=oh[:], in_=onehot[:, :])

            cw_all = res.tile([P, TT], F32)
            mask_all = res.tile([P, TT], F32)
            for n in range(T // 512):
                ps_lg = ps.tile([E, 512], F32, tag="pA", space="PSUM")
                for k in range(KH):
                    xs = sb.tile([P, 512], F32, tag="xsplit", bufs=5)
                    nc.sync.dma_start(
                        out=xs[:], in_=xT[k * P:(k + 1) * P, n * 512:(n + 1) * 512])
                    nc.tensor.matmul(
                        ps_lg[:], lhsT=gs[:, k * E:(k + 1) * E], rhs=xs[:],
                        start=(k == 0), stop=(k == KH - 1))
                lgT = sb.tile([E, 512], F32, tag="lgT")
                nc.scalar.copy(lgT[:], ps_lg[:])
                for j in range(4):
                    t = n * 4 + j
                    ps_tp = ps.tile([P, E], F32, tag="pB", space="PSUM")
                    nc.tensor.transpose(out=ps_tp[:], in_=lgT[:, j * P:(j + 1) * P],
                                        identity=idf[0:E, 0:E])
                    lg = sb.tile([P, E], F32, tag="lg")
                    nc.scalar.activation(out=lg[:], in_=ps_tp[:], func=AF.Copy,
                                         scale=rinv_all[:, t:t + 1])
                    # softmax + top2
                    ngm = sb.tile([P, 1], F32, tag="ngm")
                    nc.vector.tensor_reduce(out=ngm[:], in_=lg[:], axis=AX.X,
                                            op=OP.max, negate=True)
                    probs = sb.tile([P, E], F32, tag="probs")
                    nc.scalar.activation(out=probs[:], in_=lg[:], func=AF.Exp,
                                         bias=ngm[:])
                    top8 = sb.tile([P, E], F32, tag="top8")
                    nc.vector.max(out=top8[:], in_=probs[:])
                    den = sb.tile([P, 1], F32, tag="den")
                    nc.vector.tensor_add(out=den[:], in0=top8[:, 0:1], in1=top8[:, 1:2])
                    rden = sb.tile([P, 1], F32, tag="rden")
                    nc.vector.reciprocal(rden[:], den[:])
                    pex = sb.tile([P, E], F32, tag="pex")
                    nc.vector.tensor_mul(out=pex[:], in0=probs[:], in1=oh[:])
                    pe = sb.tile([P, 1], F32, tag="pe")
                    nc.vector.reduce_sum(out=pe[:], in_=pex[:], axis=AX.X)
                    nc.vector.tensor_tensor(out=mask_all[:, t:t + 1], in0=pe[:],
                                            in1=top8[:, 1:2], op=OP.is_ge)
                    cw0 = sb.tile([P, 1], F32, tag="cw0")
                    nc.vector.tensor_mul(out=cw0[:], in0=pe[:], in1=mask_all[:, t:t + 1])
                    nc.vector.tensor_mul(out=cw_all[:, t:t + 1], in0=cw0[:], in1=rden[:])

            # ---------------- M5: compaction ----------------
            ps_mt = ps.tile([TT, P], F32, tag="pB", space="PSUM")
            nc.tensor.transpose(out=ps_mt[:], in_=mask_all[:], identity=idf[:])
            mtp = sb.tile([TT, P], F32, tag="mtp")
            nc.scalar.copy(mtp[:], ps_mt[:])
            cs = sb.tile([TT, 1], F32, tag="cs")
            nc.vector.reduce_sum(out=cs[:], in_=mtp[:], axis=AX.X)
            ps_pos = ps.tile([P, TT], F32, tag="pA", space="PSUM")
            nc.tensor.matmul(ps_pos[:], lhsT=lt128[:], rhs=mask_all[:],
                             start=True, stop=False)
            nc.tensor.matmul(ps_pos[:], lhsT=cs[:].to_broadcast([TT, P]),
                             rhs=lt16[:], start=False, stop=True)
            slotf = sb.tile([P, TT], F32, tag="slotf")
            nc.vector.scalar_tensor_tensor(out=slotf[:], in0=ps_pos[:], scalar=4096.0,
                                           in1=mask_all[:], op0=OP.subtract, op1=OP.mult)
            nc.vector.tensor_scalar_add(slotf[:], slotf[:], 4096.0)
            # one-hot compaction: psc rows = [sum pid*oh, sum cw*oh, occ, sum t*oh]
            pid_i = sb.tile([P, 1], I32, tag="pid_i")
            nc.gpsimd.iota(pid_i[:], pattern=[[0, 1]], base=0, channel_multiplier=1)
            tv_i = sb.tile([P, TT], I32, tag="tv_i")
            nc.gpsimd.iota(tv_i[:], pattern=[[1, TT]], base=0, channel_multiplier=0)
            ic_scr = sb.tile([P, CAP], I32, tag="csb", bufs=1)
            nc.gpsimd.iota(ic_scr[:], pattern=[[1, CAP]], base=0, channel_multiplier=0)
            iotacols = cpool.tile([P, CAP], F32)
            nc.vector.tensor_copy(iotacols[:], ic_scr[:])
            lhs4 = cpool.tile([P, 4 * TT], BF16)
            lhs4v = lhs4.rearrange("p (t four) -> p t four", four=4)
            nc.vector.tensor_copy(lhs4v[:, :, 0], pid_i[:].to_broadcast([P, TT]))
            nc.vector.tensor_copy(lhs4v[:, :, 1], cw_all[:])
            nc.vector.memset(lhs4v[:, :, 2], 1.0)
            nc.vector.tensor_copy(lhs4v[:, :, 3], tv_i[:])
            psc_a = ps1.tile([4, 512], F32, tag="pd", space="PSUM")
            psc_b = ps1.tile([4, CAP - 512], F32, tag="po", space="PSUM")
            for t in range(TT):
                oh_t = sb.tile([P, CAP], BF16, tag="oh_t", bufs=2)
                nc.vector.tensor_scalar(out=oh_t[:], in0=iotacols[:],
                                        scalar1=slotf[:, t:t + 1], scalar2=None,
                                        op0=OP.is_equal)
                nc.tensor.matmul(psc_a[:], lhsT=lhs4[:, 4 * t:4 * t + 4],
                                 rhs=oh_t[:, 0:512],
                                 start=(t == 0), stop=(t == TT - 1))
                nc.tensor.matmul(psc_b[:], lhsT=lhs4[:, 4 * t:4 * t + 4],
                                 rhs=oh_t[:, 512:CAP],
                                 start=(t == 0), stop=(t == TT - 1))
            csb = sb.tile([4, CAP], F32, tag="csb", bufs=1)
            nc.scalar.copy(csb[:, 0:512], psc_a[:])
            nc.scalar.copy(csb[:, 512:CAP], psc_b[:])
            idx_i = res.tile([P, G], I32)
            cw_slots = res.tile([P, G], F32)
            for g in range(G):
                tpc = ps.tile([P, 4], F32, tag="pB", space="PSUM")
                nc.tensor.transpose(out=tpc[:], in_=csb[:, g * P:(g + 1) * P],
                                    identity=idf[0:4, 0:4])
                scr = sb.tile([P, 4], F32, tag="scr")
                nc.scalar.copy(scr[:], tpc[:])
                idxf = sb.tile([P, 1], F32, tag="idxf")
                nc.vector.scalar_tensor_tensor(out=idxf[:], in0=scr[:, 3:4],
                                               scalar=128.0, in1=scr[:, 0:1],
                                               op0=OP.mult, op1=OP.add)
                emp = sb.tile([P, 1], F32, tag="emp")
                nc.vector.tensor_scalar(out=emp[:], in0=scr[:, 2:3],
                                        scalar1=-2048.0, scalar2=2048.0,
                                        op0=OP.mult, op1=OP.add)
                nc.vector.tensor_add(out=idxf[:], in0=idxf[:], in1=emp[:])
                nc.vector.tensor_copy(idx_i[:, g:g + 1], idxf[:])
                nc.vector.tensor_copy(cw_slots[:, g:g + 1], scr[:, 1:2])

            # ---------------- M6: gather + transpose ----------------
            xgT = [res.tile([P, CAP], BF16, tag=f"xgT{k}", name=f"xgT{k}") for k in range(KH)]
            for g in range(G):
                gx = sb.tile([P, H], BF16, tag="gx")
                nc.vector.memset(gx[:], 0.0)
                nc.gpsimd.indirect_dma_start(
                    out=gx[:], out_offset=None, in_=xnorm_d[:, :],
                    in_offset=bass.IndirectOffsetOnAxis(ap=idx_i[:, g:g + 1], axis=0),
                    bounds_check=T - 1, oob_is_err=False)
                for k in range(KH):
                    ps_tp = ps.tile([P, P], BF16, tag="pB", space="PSUM")
                    nc.tensor.transpose(out=ps_tp[:], in_=gx[:, k * P:(k + 1) * P],
                                        identity=idb[:])
                    nc.scalar.copy(xgT[k][:, g * P:(g + 1) * P], ps_tp[:])

            # Residual-MLP weight preloads (issued interleaved with the FFN
            # streams so they complete before the ReduceScatter hogs the DMA
            # queues; recycled tags keep SBUF flat).
            _ptags = ["wqh", "wqh", "ropeqf",
                      "xsplit", "xsplit", "xsplit", "xsplit", "xsplit",
                      "xn", "xn", "sqs", "sqs", "xns", "xns", "gx", "gx",
                      "srk", "srk", "xkvS", "xkvS", "sqk", "sqk", "lgT", "lgT"]
            _psrc = [(rw1, m) for m in range(KH)] + [(rw3, m) for m in range(KH)] \
                    + [(rw2, m) for m in range(KH)]
            rwpre = []

            def emit_preload():
                i_ = len(rwpre)
                if i_ >= len(_psrc):
                    return
                wsrc, m = _psrc[i_]
                _tg = _ptags[i_]
                _bufs = {"xsplit": 5}.get(_tg, 2)
                tt_ = sb.tile([P, H], BF16, tag=_tg, name=f"rwpre{i_}", bufs=_bufs)
                nc.sync.dma_start(
                    out=tt_[:],
                    in_=wsrc[m, :, :])
                rwpre.append(tt_)

            # ---------------- M7: expert FFN on CAP slots ----------------
            NSC = ((0, 512), (512, CAP - 512))
            hT = [res.tile([P, CAP], BF16, tag=f"hT{m}", name=f"hT{m}") for m in range(FM)]
            for m in range(FM):
                w1m = sb.tile([P, H], BF16, tag="w1m", bufs=2)
                nc.sync.dma_start(
                    out=w1m[:],
                    in_=ew1[m, :, :])
                w3m = sb.tile([P, H], BF16, tag="w3m", bufs=2)
                nc.sync.dma_start(
                    out=w3m[:],
                    in_=ew3[m, :, :])
                p1a = ps.tile([P, 512], F32, tag="pA", space="PSUM", name="p1a")
                p1b = ps.tile([P, 128], F32, tag="pA", space="PSUM", name="p1b")
                p3a = ps.tile([P, 512], F32, tag="pB", space="PSUM", name="p3a")
                p3b = ps.tile([P, 128], F32, tag="pB", space="PSUM", name="p3b")
                for k in range(KH):
                    st, sp = k == 0, k == KH - 1
                    nc.tensor.matmul(p1a[:], lhsT=w1m[:, k * P:(k + 1) * P],
                                     rhs=xgT[k][:, 0:512], start=st, stop=sp)
                    nc.tensor.matmul(p1b[:], lhsT=w1m[:, k * P:(k + 1) * P],
                                     rhs=xgT[k][:, 512:CAP], start=st, stop=sp)
                for k in range(KH):
                    st, sp = k == 0, k == KH - 1
                    nc.tensor.matmul(p3a[:], lhsT=w3m[:, k * P:(k + 1) * P],
                                     rhs=xgT[k][:, 0:512], start=st, stop=sp)
                    nc.tensor.matmul(p3b[:], lhsT=w3m[:, k * P:(k + 1) * P],
                                     rhs=xgT[k][:, 512:CAP], start=st, stop=sp)
                emit_preload()
                for ns0, nsw, p1, p3 in ((0, 512, p1a, p3a), (512, CAP - 512, p1b, p3b)):
                    t1 = sb.tile([P, 512], BF16, tag="t1", name="t1")
                    nc.scalar.activation(out=t1[:, :nsw], in_=p1[:, :nsw], func=AF.Sigmoid)
                    tb = sb.tile([P, 512], BF16, tag="tb", name="tb")
                    nc.vector.tensor_tensor(out=tb[:, :nsw], in0=t1[:, :nsw],
                                            in1=p1[:, :nsw], op=OP.mult)
                    nc.vector.tensor_tensor(out=hT[m][:, ns0:ns0 + nsw],
                                            in0=tb[:, :nsw], in1=p3[:, :nsw], op=OP.mult)
            ynat = [res.tile([P, H], BF16, tag=f"ynat{g}", name=f"ynat{g}") for g in range(G)]
            for mh in range(KH):
                w2a = sb.tile([P, 11 * P], BF16, tag="w2m", bufs=2, name="w2a")
                nc.sync.dma_start(
                    out=w2a[:],
                    in_=ew2[mh, :, 0:11 * P])
                w2b = sb.tile([P, 11 * P], BF16, tag="w2m", bufs=2, name="w2b")
                nc.sync.dma_start(
                    out=w2b[:],
                    in_=ew2[mh, :, 11 * P:F])
                yT = sb.tile([P, CAP], BF16, tag="yT")
                pya = ps.tile([P, 512], F32, tag="pA", space="PSUM", name="pya")
                pyb = ps.tile([P, 128], F32, tag="pB", space="PSUM", name="pyb")
                for k in range(FM):
                    wsrc = w2a if k < 11 else w2b
                    lhs = wsrc[:, (k % 11) * P:(k % 11 + 1) * P]
                    st, sp = k == 0, k == FM - 1
                    nc.tensor.matmul(pya[:], lhsT=lhs, rhs=hT[k][:, 0:512],
                                     start=st, stop=sp)
                    nc.tensor.matmul(pyb[:], lhsT=lhs, rhs=hT[k][:, 512:CAP],
                                     start=st, stop=sp)
                nc.scalar.copy(yT[:, 0:512], pya[:])
                nc.scalar.copy(yT[:, 512:CAP], pyb[:])
                emit_preload()
                for g in range(G):
                    ps_tp = ps.tile([P, P], BF16, tag="pB", space="PSUM")
                    nc.tensor.transpose(out=ps_tp[:], in_=yT[:, g * P:(g + 1) * P],
                                        identity=idb[:])
                    nc.scalar.activation(out=ynat[g][:, mh * P:(mh + 1) * P],
                                         in_=ps_tp[:], func=AF.Copy,
                                         scale=cw_slots[:, g:g + 1])
            for t in range(TT):
                nc.sync.dma_start(out=acc_d[t * P:(t + 1) * P, :], in_=zb[:])
            for g in range(G):
                nc.gpsimd.indirect_dma_start(
                    out=acc_d[:, :],
                    out_offset=bass.IndirectOffsetOnAxis(ap=idx_i[:, g:g + 1], axis=0),
                    in_=ynat[g][:], in_offset=None,
                    bounds_check=T - 1, oob_is_err=False)

            rw1p, rw3p, rw2p = rwpre[0:KH], rwpre[KH:2 * KH], rwpre[2 * KH:3 * KH]

            # D3: attention per head (all tiles base-partition 0)
            mk = [cpool.tile([P, TS], F32, name=f"mk{c}") for c in range(4)]
            for c in range(4):
                nc.sync.dma_start(out=mk[c][:], in_=maskT[c * P:(c + 1) * P, :])
            attnh = [res.tile([HD, TS], BF16, tag=(f"attnh{h}" if h < 2 else f"qrh{h - 2}"), name=f"attnh{h}") for h in range(NH)]
            for h in range(NH):
                kvh = h // 4
                pd = ps1.tile([P, TS], F32, tag="pd", space="PSUM")
                po = ps1.tile([HD, TS], F32, tag="po", space="PSUM")
                for c in range(4):
                    ps_s = ps.tile([P, TS], F32, tag="pC", space="PSUM")
                    nc.tensor.matmul(ps_s[:],
                                     lhsT=krh[kvh][:, c * P:(c + 1) * P],
                                     rhs=qrh[h][:], start=True, stop=True)
                    sm = sb.tile([P, TS], F32, tag="sm")
                    nc.vector.tensor_add(out=sm[:], in0=ps_s[:], in1=mk[c][:])
                    pT = sb.tile([P, TS], BF16, tag="pT", bufs=3)
                    nc.scalar.activation(out=pT[:], in_=sm[:], func=AF.Exp, scale=0.125)
                    nc.tensor.matmul(pd[:], lhsT=ones_b[:], rhs=pT[:],
                                     start=(c == 0), stop=(c == 3))
                    nc.tensor.matmul(po[:], lhsT=vnat[c][:, kvh * HD:(kvh + 1) * HD],
                                     rhs=pT[:], start=(c == 0), stop=(c == 3))
                rd = sb.tile([HD, TS], F32, tag="rd")
                nc.vector.reciprocal(rd[:], pd[0:HD, :])
                nc.vector.tensor_tensor(out=attnh[h][:], in0=po[:],
                                        in1=rd[:], op=OP.mult)

            # D4: output projection (contraction in 16 chunks of 64) + residual
            RAT = [res.tile([P, TS], F32, tag=f"xgT{m}", name=f"RAT{m}") for m in range(KH)]
            for m in range(KH):
                woa = sb.tile([HD, 8 * P], BF16, tag="wom", bufs=2, name="woa")
                nc.sync.dma_start(
                    out=woa[:],
                    in_=wo[m, :, 0:8 * P])
                wob = sb.tile([HD, 8 * P], BF16, tag="wom", bufs=2, name="wob")
                nc.sync.dma_start(
                    out=wob[:],
                    in_=wo[m, :, 8 * P:NH * P])
                op_ps = ps.tile([P, TS], F32, tag="pB", space="PSUM")
                for k in range(NH):
                    wsrc = woa if k < 8 else wob
                    nc.tensor.matmul(op_ps[:], lhsT=wsrc[:, (k % 8) * P:(k % 8 + 1) * P],
                                     rhs=attnh[k][:], start=(k == 0), stop=(k == NH - 1))
                xres = sb.tile([P, TS], F32, tag="xres", name="xres")
                nc.sync.dma_start(out=xres[:], in_=xT_kv[m * P:(m + 1) * P, TS:KV])
                nc.vector.tensor_add(out=RAT[m][:], in0=op_ps[:], in1=xres[:])

            # D5: residual MLP
            ps_rm = ps.tile([P, TS], F32, tag="pA", space="PSUM")
            for m in range(KH):
                sqm = sb.tile([P, TS], BF16, tag="sqm")
                nc.scalar.activation(out=sqm[:], in_=RAT[m][:], func=AF.Square)
                nc.tensor.matmul(ps_rm[:], lhsT=ones_b[:], rhs=sqm[:],
                                 start=(m == 0), stop=(m == KH - 1))
            srm = sb.tile([P, TS], F32, tag="srm")
            nc.scalar.activation(out=srm[:], in_=ps_rm[:], func=AF.Sqrt,
                                 scale=1.0 / H, bias=epsb[:])
            rrm = sb.tile([P, TS], F32, tag="rrm", bufs=1)
            nc.vector.reciprocal(rrm[:], srm[:])
            xmT = [res.tile([P, TS], BF16, tag=f"hT{16 + m}" if m < 6 else f"ynat{m - 6}", name=f"xmT{m}") for m in range(KH)]
            for m in range(KH):
                nc.vector.tensor_mul(out=xmT[m][:], in0=RAT[m][:], in1=rrm[:])
            hm = [res.tile([P, TS], BF16, tag=f"hT{8 + m}", name=f"hm{m}") for m in range(KH)]
            for m in range(KH):
                p1 = ps.tile([P, TS], F32, tag="pB", space="PSUM")
                for k in range(KH):
                    nc.tensor.matmul(p1[:], lhsT=rw1p[m][:, k * P:(k + 1) * P],
                                     rhs=xmT[k][:], start=(k == 0), stop=(k == KH - 1))
                p3 = ps.tile([P, TS], F32, tag="pC", space="PSUM")
                for k in range(KH):
                    nc.tensor.matmul(p3[:], lhsT=rw3p[m][:, k * P:(k + 1) * P],
                                     rhs=xmT[k][:], start=(k == 0), stop=(k == KH - 1))
                t1 = sb.tile([P, TS], BF16, tag="t1d")
                nc.scalar.activation(out=t1[:], in_=p1[:], func=AF.Sigmoid)
                tb = sb.tile([P, TS], BF16, tag="tbd")
                nc.vector.tensor_tensor(out=tb[:], in0=t1[:], in1=p1[:], op=OP.mult)
                nc.vector.tensor_tensor(out=hm[m][:], in0=tb[:], in1=p3[:], op=OP.mult)

            # D6a: rw2 + residual accumulated in place into RAT (pre-collective)
            for m in range(KH):
                p2 = ps.tile([P, TS], F32, tag="pB", space="PSUM")
                for k in range(KH):
                    nc.tensor.matmul(p2[:], lhsT=rw2p[m][:, k * P:(k + 1) * P],
                                     rhs=hm[k][:], start=(k == 0), stop=(k == KH - 1))
                nc.vector.tensor_add(out=RAT[m][:], in0=p2[:], in1=RAT[m][:])
            # ---------------- M8: ReduceScatter ----------------
            nc.gpsimd.collective_compute(
                "ReduceScatter", OP.add, replica_groups=[list(range(NCORES))],
                ins=[acc_d.ap().opt()], outs=[rs_d.ap().opt()])

            # D6b: MoE slice transpose + final sum -> output
            moeT = [res.tile([P, TS], F32, tag=f"hT{k}", name=f"moeT{k}") for k in range(KH)]
            for pt in range(2):
                rsb = sb.tile([P, H], BF16, tag="rsb")
                nc.sync.dma_start(out=rsb[:], in_=rs_d[pt * P:(pt + 1) * P, :])
                for k in range(KH):
                    ps_tp = ps.tile([P, P], BF16, tag="pB", space="PSUM")
                    nc.tensor.transpose(out=ps_tp[:], in_=rsb[:, k * P:(k + 1) * P],
                                        identity=idb[:])
                    nc.scalar.copy(moeT[k][:, pt * P:(pt + 1) * P], ps_tp[:])
            for m in range(KH):
                ot = sb.tile([P, TS], F32, tag="ot")
                nc.vector.tensor_add(out=ot[:], in0=RAT[m][:], in1=moeT[m][:])
                nc.sync.dma_start(out=out[m * P:(m + 1) * P, :], in_=ot[:])

    nc.finalize()
    _BUILD_CACHE["nc"] = nc
    return nc


def _host_prep(inputs):
    f32 = np.float32
    x = np.asarray(inputs["hidden_states"], f32).reshape(T, H)
    ln1 = np.asarray(inputs["ln1_w"], f32)
    res_ln = np.asarray(inputs["res_ln_w"], f32)
    post_ln = np.asarray(inputs["post_ln_w"], f32)

    import ml_dtypes
    bf16 = ml_dtypes.bfloat16

    def b(a):
        return np.ascontiguousarray(np.asarray(a, f32)).astype(bf16)

    def mmaj(w, pp, mm):
        # [K, M] -> [M//mm, pp, (K//pp)*mm] with w[k, m] at [m//mm, k%pp, (k//pp)*mm + m%mm]
        K, M = w.shape
        return np.ascontiguousarray(
            w.reshape(K // pp, pp, M // mm, mm).transpose(2, 1, 0, 3).reshape(M // mm, pp, (K // pp) * mm))

    wq = mmaj(b(ln1[:, None] * np.asarray(inputs["q_w"], f32)), 128, 64)
    wk = mmaj(b(ln1[:, None] * np.asarray(inputs["k_w"], f32)), 128, 64)
    wv = mmaj(b(ln1[:, None] * np.asarray(inputs["v_w"], f32)), 128, 128)
    wo = mmaj(b(inputs["o_w"]), 64, 128)
    rw1 = mmaj(b(res_ln[:, None] * np.asarray(inputs["rw1"], f32)), 128, 128)
    rw3 = mmaj(b(res_ln[:, None] * np.asarray(inputs["rw3"], f32)), 128, 128)
    rw2 = mmaj(b(inputs["rw2"]), 128, 128)
    gate = np.ascontiguousarray(post_ln[:, None] * np.asarray(inputs["gate_w"], f32))
    gatep = np.ascontiguousarray(gate.reshape(8, 128, 8).transpose(1, 0, 2).reshape(128, 64))
    xT = np.ascontiguousarray(x.T)                       # [H, T]

    e_w1 = np.asarray(inputs["e_w1"], f32)
    e_w3 = np.asarray(inputs["e_w3"], f32)
    e_w2 = np.asarray(inputs["e_w2"], f32)

    # RoPE tables: cos64[d, pos] with d in [0,64), duplicated inv-freq halves
    pos = np.arange(S, dtype=f32)
    inv = 1.0 / (THETA ** (np.arange(0, HD, 2, dtype=f32) / HD))   # [32]
    ang = inv[:, None] * pos[None, :]                               # [32, S]
    cos64 = np.concatenate([np.cos(ang)] * 2, 0)                    # [64, S]
    sin64 = np.concatenate([np.sin(ang)] * 2, 0)

    in_maps = []
    for core in range(NCORES):
        bi, c = divmod(core, 4)
        lo = bi * S + c * TS
        # kv window: previous chunk + own chunk (zeros for c == 0)
        xkv = np.zeros((H, KV), f32)
        if c > 0:
            xkv[:, :TS] = xT[:, lo - TS:lo]
        xkv[:, TS:] = xT[:, lo:lo + TS]
        # mask: valid iff ql < kl <= ql + TS (and kl >= TS when c == 0)
        ql = np.arange(TS)[None, :]
        kl = np.arange(KV)[:, None]
        valid = (kl > ql) & (kl <= ql + TS)
        if c == 0:
            valid &= kl >= TS
        maskT = np.where(valid, 0.0, NEG).astype(f32)
        # RoPE positions (within-sequence)
        pq = c * TS + np.arange(TS)
        pk = np.clip((c - 1) * TS + np.arange(KV), 0, S - 1)
        cq = np.tile(cos64[:, pq], (2, 1)).astype(f32)
        sqv = np.tile(sin64[:, pq], (2, 1)).astype(f32)
        ckv = np.tile(cos64[:, pk], (2, 1)).astype(f32)
        skv = np.tile(sin64[:, pk], (2, 1)).astype(f32)
        oh = np.zeros((P, E), f32)
        oh[:, core] = 1.0
        in_maps.append(dict(
            xT_kv=xkv, xnat=x.astype(bf16), xT=xT, gatep=gatep, onehot=oh,
            cos_q=cq, sin_q=sqv, cos_k=ckv, sin_k=skv, maskT=maskT,
            wq=wq, wk=wk, wv=wv, wo=wo, rw1=rw1, rw3=rw3, rw2=rw2,
            ew1=mmaj(b(post_ln[:, None] * e_w1[core]), 128, 128),
            ew3=mmaj(b(post_ln[:, None] * e_w3[core]), 128, 128),
            ew2=mmaj(b(e_w2[core]), 128, 128),
        ))
    return in_maps


def kernel(**inputs) -> np.ndarray:
    nc = _build()
    in_maps = _host_prep(inputs)
    res = run_bass_kernel_spmd(nc, in_maps, core_ids=list(range(NCORES)))
    outs = [np.asarray(res.results[i]["out"], np.float32).T for i in range(NCORES)]
    full = np.concatenate(outs, 0)          # [T, H] in core order == token order
    return full.reshape(B, S, H)


# revision 35
# speedup vs baseline: 1.1142x; 1.0532x over previous
"""Arctic decoder layer (attention + residual MLP + top-2 MoE) on 8 TRN2 NeuronCores.

Strategy:
  - Data parallel over tokens for attention/norms/residual MLP (256 tokens/core,
    sliding-window attention needs only the previous 256-token chunk as halo).
  - Expert parallel for the MoE: every core receives the full (replicated) input,
    computes gating for all 2048 tokens, compacts the token indices routed to ITS
    expert (capacity 640), gathers them with indirect DMA, runs the expert FFN on
    the gathered tokens only, scales by the combine weights and scatters into a
    zeroed [2048, 1024] accumulator; one ReduceScatter(add) returns each core its
    own 256-token slice of the MoE output.
  - Activations live as [feature, token] (transposed) for matmuls; natural
    [token, feature] layout is used for RMS statistics, gating softmax/top-2 and
    the gather/scatter.  Matmuls run in bf16 (f32 PSUM accumulation); the gating
    logits use a bf16 split-float (hi+lo) product to keep top-2 selection exact.
"""
import os
import sys

for _p in ("/opt/trn_rl_repo", "/root/.axon_site/_ro/trn_rl_repo", "/root/.axon_site"):
    if os.path.isdir(_p) and _p not in sys.path:
        sys.path.append(_p)

import numpy as np

import concourse.bass as bass
import concourse.bacc as bacc
import concourse.mybir as mybir
import concourse.tile as tile
from concourse.bass_utils import run_bass_kernel_spmd
from concourse.masks import make_identity

F32 = mybir.dt.float32
BF16 = mybir.dt.bfloat16
I32 = mybir.dt.int32
AF = mybir.ActivationFunctionType
OP = mybir.AluOpType
AX = mybir.AxisListType

NCORES = 8
P = 128
B, S, H = 2, 1024, 1024
T = B * S                 # 2048 tokens
TT = T // P               # 16 token tiles
KH = H // P               # 8 hidden k-chunks
NH, NKV, HD = 16, 4, 64
F = 2816
FM = F // P               # 22
E = 8
CAP = 640                 # per-expert token capacity (actual max load is ~531)
G = CAP // P              # 5 slot batches
TS = T // NCORES          # 256 tokens per core
KV = 2 * TS               # 512 kv-window tokens per core
EPS = 1e-5
THETA = 10000.0
NEG = -1.0e5              # additive mask value (pre-exp)

_BUILD_CACHE = {}


def _build():
    if "nc" in _BUILD_CACHE:
        return _BUILD_CACHE["nc"]
    nc = bacc.Bacc("TRN2", target_bir_lowering=False, debug=False, num_devices=NCORES)

    dp = nc.declare_dram_parameter
    xT_kv = dp("xT_kv", [H, KV], F32, isOutput=False)
    xnat = dp("xnat", [T, H], BF16, isOutput=False)
    xT = dp("xT", [H, T], F32, isOutput=False)
    onehot = dp("onehot", [P, E], F32, isOutput=False)
    cos_q = dp("cos_q", [P, TS], F32, isOutput=False)
    sin_q = dp("sin_q", [P, TS], F32, isOutput=False)
    cos_k = dp("cos_k", [P, KV], F32, isOutput=False)
    sin_k = dp("sin_k", [P, KV], F32, isOutput=False)
    maskT = dp("maskT", [KV, TS], F32, isOutput=False)
    wq = dp("wq", [NH, P, KH * HD], BF16, isOutput=False)
    wk = dp("wk", [NKV, P, KH * HD], BF16, isOutput=False)
    wv = dp("wv", [2, P, H], BF16, isOutput=False)
    wo = dp("wo", [KH, HD, NH * P], BF16, isOutput=False)
    rw1 = dp("rw1", [KH, P, H], BF16, isOutput=False)
    rw3 = dp("rw3", [KH, P, H], BF16, isOutput=False)
    rw2 = dp("rw2", [KH, P, H], BF16, isOutput=False)
    ew1 = dp("ew1", [FM, P, H], BF16, isOutput=False)
    ew3 = dp("ew3", [FM, P, H], BF16, isOutput=False)
    ew2 = dp("ew2", [KH, P, F], BF16, isOutput=False)
    gatep = dp("gatep", [P, KH * E], F32, isOutput=False)
    out = dp("out", [H, TS], F32, isOutput=True)

    # internal DRAM (offset-0 targets for indirect DMA + collective bounces)
    xnorm_d = nc.dram_tensor("xnorm_d", [T, H], BF16)
    acc_d = nc.dram_tensor("acc_d", [T, H], BF16)
    rs_d = nc.dram_tensor("rs_d", [TS, H], BF16)

    with tile.TileContext(nc) as tc:
        with (
            tc.tile_pool(name="const", bufs=1) as cpool,
            tc.tile_pool(name="sb", bufs=2) as sb,
            tc.tile_pool(name="res", bufs=1) as res,
            tc.tile_pool(name="ps", bufs=2, space="PSUM") as ps,
            tc.tile_pool(name="ps1", bufs=1, space="PSUM") as ps1,
        ):
            # ---------------- constants ----------------
            idf = cpool.tile([P, P], F32)
            make_identity(nc, idf[:])
            idb = cpool.tile([P, P], BF16)
            make_identity(nc, idb[:])
            ones_b = cpool.tile([P, P], BF16)
            nc.vector.memset(ones_b[:], 1.0)
            # strict lower-triangular LT[k, m] = 1 if k < m (for exclusive cumsum)
            lt128 = cpool.tile([P, P], F32)
            nc.gpsimd.memset(lt128[:], 0.0)
            nc.gpsimd.affine_select(out=lt128[:], in_=lt128[:], pattern=[[-1, P]],
                                    compare_op=OP.is_ge, fill=1.0, base=0,
                                    channel_multiplier=1)
            lt16 = cpool.tile([TT, TT], F32)
            nc.gpsimd.memset(lt16[:], 0.0)
            nc.gpsimd.affine_select(out=lt16[:], in_=lt16[:], pattern=[[-1, TT]],
                                    compare_op=OP.is_ge, fill=1.0, base=0,
                                    channel_multiplier=1)
            # signed rotate-half permutation for RoPE: rot[m] = -q[m+32] | q[m-32]
            r64 = np.zeros((HD, HD), np.float32)
            for mm in range(32):
                r64[mm + 32, mm] = -1.0
                r64[mm, mm + 32] = 1.0
            r64_d = nc.inline_tensor(r64, name="r64_const")
            r64t = cpool.tile([HD, HD], F32)
            nc.sync.dma_start(out=r64t[:], in_=r64_d[:, :])
            epsb = cpool.tile([P, 1], F32)
            nc.vector.memset(epsb[:], EPS)
            zb = cpool.tile([P, H], BF16)
            nc.vector.memset(zb[:], 0.0)
            for t in range(TT):
                nc.sync.dma_start(out=acc_d[t * P:(t + 1) * P, :], in_=zb[:])

            # ================= DP path (overlaps the collective) =============
            # D1: RMS over the 512-token kv window (transposed layout)
            ps_rms = ps.tile([P, KV], F32, tag="pA", space="PSUM")
            for k in range(KH):
                xk1 = sb.tile([P, KV], F32, tag="xkvS", name="xk1")
                nc.sync.dma_start(out=xk1[:], in_=xT_kv[k * P:(k + 1) * P, :])
                sqk = sb.tile([P, KV], BF16, tag="sqk")
                nc.scalar.activation(out=sqk[:], in_=xk1[:], func=AF.Square)
                nc.tensor.matmul(ps_rms[:], lhsT=ones_b[:], rhs=sqk[:],
                                 start=(k == 0), stop=(k == KH - 1))
            srk = sb.tile([P, KV], F32, tag="srk")
            nc.scalar.activation(out=srk[:], in_=ps_rms[:], func=AF.Sqrt,
                                 scale=1.0 / H, bias=epsb[:])
            rkv = sb.tile([P, KV], F32, tag="rkv", bufs=1)
            nc.vector.reciprocal(rkv[:], srk[:])
            xnkv = [res.tile([P, KV], BF16, tag=f"xnkv{k}", name=f"xnkv{k}") for k in range(KH)]
            for k in range(KH):
                xk2 = sb.tile([P, KV], F32, tag="xkvS", name="xk2")
                nc.sync.dma_start(out=xk2[:], in_=xT_kv[k * P:(k + 1) * P, :])
                nc.vector.tensor_mul(out=xnkv[k][:], in0=xk2[:], in1=rkv[:])

            # D2: q/k/v projections + RoPE + v transpose
            cq = cpool.tile([P, TS], F32)
            nc.sync.dma_start(out=cq[:], in_=cos_q[:, :])
            sq = cpool.tile([P, TS], F32)
            nc.sync.dma_start(out=sq[:], in_=sin_q[:, :])
            ck = cpool.tile([P, KV], F32)
            nc.sync.dma_start(out=ck[:], in_=cos_k[:, :])
            sk = cpool.tile([P, KV], F32)
            nc.sync.dma_start(out=sk[:], in_=sin_k[:, :])

            def rope(src_ps, cos_t, sin_t, w, dst, tagsfx):
                # src_ps: [HD, w] psum f32 (one head); dst: [HD, w] bf16 sbuf
                qf = sb.tile([HD, KV], F32, tag="ropeqf", name="ropeqf")
                nc.scalar.copy(qf[:, :w], src_ps[:, :w])
                rot = ps.tile([HD, KV], F32, tag="pC", space="PSUM", name="roperot")
                nc.tensor.matmul(rot[:, :w], lhsT=r64t[:], rhs=qf[:, :w],
                                 start=True, stop=True)
                t1 = sb.tile([HD, KV], F32, tag="ropet1", name="ropet1")
                nc.vector.tensor_mul(out=t1[:, :w], in0=qf[:, :w], in1=cos_t[0:HD, :w])
                nc.vector.tensor_mul(out=dst, in0=rot[:, :w], in1=sin_t[0:HD, :w])
                nc.vector.tensor_add(out=dst, in0=t1[:, :w], in1=dst)

            # per-head q (16 x [64, 256]) and per-kv-head k (4 x [64, 512])
            qrh = [res.tile([HD, TS], BF16, tag=f"qrh{h}", name=f"qrh{h}") for h in range(NH)]
            for h in range(NH):
                wqh = sb.tile([P, KH * HD], BF16, tag="wqh")
                nc.sync.dma_start(
                    out=wqh[:],
                    in_=wq[h, :, :])
                qp = ps.tile([HD, TS], F32, tag="pB", space="PSUM")
                for k in range(KH):
                    nc.tensor.matmul(qp[:], lhsT=wqh[:, k * HD:(k + 1) * HD],
                                     rhs=xnkv[k][:, TS:KV],
                                     start=(k == 0), stop=(k == KH - 1))
                rope(qp, cq, sq, TS, qrh[h][:], "q")
            krh = [res.tile([HD, KV], BF16, tag=f"krh{h}", name=f"krh{h}") for h in range(NKV)]
            vnat = [res.tile([P, NKV * HD], BF16, tag=f"vnat{c}", name=f"vnat{c}") for c in range(4)]
            for h in range(NKV):
                wkh = sb.tile([P, KH * HD], BF16, tag="wqh")
                nc.sync.dma_start(
                    out=wkh[:],
                    in_=wk[h, :, :])
                kp = ps.tile([HD, KV], F32, tag="pA", space="PSUM")
                for k in range(KH):
                    nc.tensor.matmul(kp[:], lhsT=wkh[:, k * HD:(k + 1) * HD],
                                     rhs=xnkv[k][:],
                                     start=(k == 0), stop=(k == KH - 1))
                rope(kp, ck, sk, KV, krh[h][:], "k")
            for m in range(2):
                wvm = sb.tile([P, H], BF16, tag="wqh")
                nc.sync.dma_start(
                    out=wvm[:],
                    in_=wv[m, :, :])
                vp = ps.tile([P, KV], F32, tag="pA", space="PSUM")
                for k in range(KH):
                    nc.tensor.matmul(vp[:], lhsT=wvm[:, k * P:(k + 1) * P],
                                     rhs=xnkv[k][:],
                                     start=(k == 0), stop=(k == KH - 1))
                vT = sb.tile([P, KV], BF16, tag="vT")
                nc.scalar.copy(vT[:], vp[:])
                for c in range(4):
                    ps_tp = ps.tile([P, P], BF16, tag="pB", space="PSUM")
                    nc.tensor.transpose(out=ps_tp[:], in_=vT[:, c * P:(c + 1) * P],
                                        identity=idb[:])
                    nc.scalar.copy(vnat[c][:, m * P:(m + 1) * P], ps_tp[:])


            # ---------------- M1: natural RMS over all tokens ----------------
            rinv_all = res.tile([P, TT], F32)
            for t in range(TT):
                xn = sb.tile([P, H], BF16, tag="xn")
                nc.sync.dma_start(out=xn[:], in_=xnat[t * P:(t + 1) * P, :])
                sqs = sb.tile([P, H], BF16, tag="sqs")
                ssq = sb.tile([P, 1], F32, tag="ssq")
                nc.scalar.activation(out=sqs[:], in_=xn[:], func=AF.Square,
                                     accum_out=ssq[:])
                srt = sb.tile([P, 1], F32, tag="srt")
                nc.scalar.activation(out=srt[:], in_=ssq[:], func=AF.Sqrt,
                                     scale=1.0 / H, bias=epsb[:])
                nc.vector.reciprocal(rinv_all[:, t:t + 1], srt[:])
                xns = sb.tile([P, H], BF16, tag="xns")
                nc.scalar.activation(out=xns[:], in_=xn[:], func=AF.Copy,
                                     scale=rinv_all[:, t:t + 1])
                nc.sync.dma_start(out=xnorm_d[t * P:(t + 1) * P, :], in_=xns[:])

            # ---------------- M2+M3+M4: gating ----------------
            gs = cpool.tile([P, KH * E], F32)
            nc.sync.dma_start(out=gs[:], in_=gatep[:, :])
            oh = cpool.tile([P, E], F32)
            nc.sync.dma_start(out=oh[:], in_=onehot[:, :])

            cw_all = res.tile([P, TT], F32)
            mask_all = res.tile([P, TT], F32)
            for n in range(T // 512):
                ps_lg = ps.tile([E, 512], F32, tag="pA", space="PSUM")
                for k in range(KH):
                    xs = sb.tile([P, 512], F32, tag="xsplit", bufs=5)
                    nc.sync.dma_start(
                        out=xs[:], in_=xT[k * P:(k + 1) * P, n * 512:(n + 1) * 512])
                    nc.tensor.matmul(
                        ps_lg[:], lhsT=gs[:, k * E:(k + 1) * E], rhs=xs[:],
                        start=(k == 0), stop=(k == KH - 1))
                lgT = sb.tile([E, 512], F32, tag="lgT")
                nc.scalar.copy(lgT[:], ps_lg[:])
                for j in range(4):
                    t = n * 4 + j
                    ps_tp = ps.tile([P, E], F32, tag="pB", space="PSUM")
                    nc.tensor.transpose(out=ps_tp[:], in_=lgT[:, j * P:(j + 1) * P],
                                        identity=idf[0:E, 0:E])
                    lg = sb.tile([P, E], F32, tag="lg")
                    nc.scalar.activation(out=lg[:], in_=ps_tp[:], func=AF.Copy,
                                         scale=rinv_all[:, t:t + 1])
                    # softmax + top2
                    ngm = sb.tile([P, 1], F32, tag="ngm")
                    nc.vector.tensor_reduce(out=ngm[:], in_=lg[:], axis=AX.X,
                                            op=OP.max, negate=True)
                    probs = sb.tile([P, E], F32, tag="probs")
                    nc.scalar.activation(out=probs[:], in_=lg[:], func=AF.Exp,
                                         bias=ngm[:])
                    top8 = sb.tile([P, E], F32, tag="top8")
                    nc.vector.max(out=top8[:], in_=probs[:])
                    den = sb.tile([P, 1], F32, tag="den")
                    nc.vector.tensor_add(out=den[:], in0=top8[:, 0:1], in1=top8[:, 1:2])
                    rden = sb.tile([P, 1], F32, tag="rden")
                    nc.vector.reciprocal(rden[:], den[:])
                    pex = sb.tile([P, E], F32, tag="pex")
                    nc.vector.tensor_mul(out=pex[:], in0=probs[:], in1=oh[:])
                    pe = sb.tile([P, 1], F32, tag="pe")
                    nc.vector.reduce_sum(out=pe[:], in_=pex[:], axis=AX.X)
                    nc.vector.tensor_tensor(out=mask_all[:, t:t + 1], in0=pe[:],
                                            in1=top8[:, 1:2], op=OP.is_ge)
                    cw0 = sb.tile([P, 1], F32, tag="cw0")
                    nc.vector.tensor_mul(out=cw0[:], in0=pe[:], in1=mask_all[:, t:t + 1])
                    nc.vector.tensor_mul(out=cw_all[:, t:t + 1], in0=cw0[:], in1=rden[:])

            # ---------------- M5: compaction ----------------
            ps_mt = ps.tile([TT, P], F32, tag="pB", space="PSUM")
            nc.tensor.transpose(out=ps_mt[:], in_=mask_all[:], identity=idf[:])
            mtp = sb.tile([TT, P], F32, tag="mtp")
            nc.scalar.copy(mtp[:], ps_mt[:])
            cs = sb.tile([TT, 1], F32, tag="cs")
            nc.vector.reduce_sum(out=cs[:], in_=mtp[:], axis=AX.X)
            ps_pos = ps.tile([P, TT], F32, tag="pA", space="PSUM")
            nc.tensor.matmul(ps_pos[:], lhsT=lt128[:], rhs=mask_all[:],
                             start=True, stop=False)
            nc.tensor.matmul(ps_pos[:], lhsT=cs[:].to_broadcast([TT, P]),
                             rhs=lt16[:], start=False, stop=True)
            slotf = sb.tile([P, TT], F32, tag="slotf")
            nc.vector.scalar_tensor_tensor(out=slotf[:], in0=ps_pos[:], scalar=4096.0,
                                           in1=mask_all[:], op0=OP.subtract, op1=OP.mult)
            nc.vector.tensor_scalar_add(slotf[:], slotf[:], 4096.0)
            # one-hot compaction: psc rows = [sum pid*oh, sum cw*oh, occ, sum t*oh]
            pid_i = sb.tile([P, 1], I32, tag="pid_i")
            nc.gpsimd.iota(pid_i[:], pattern=[[0, 1]], base=0, channel_multiplier=1)
            tv_i = sb.tile([P, TT], I32, tag="tv_i")
            nc.gpsimd.iota(tv_i[:], pattern=[[1, TT]], base=0, channel_multiplier=0)
            ic_scr = sb.tile([P, CAP], I32, tag="csb", bufs=1)
            nc.gpsimd.iota(ic_scr[:], pattern=[[1, CAP]], base=0, channel_multiplier=0)
            iotacols = cpool.tile([P, CAP], F32)
            nc.vector.tensor_copy(iotacols[:], ic_scr[:])
            lhs4 = cpool.tile([P, 4 * TT], BF16)
            lhs4v = lhs4.rearrange("p (t four) -> p t four", four=4)
            nc.vector.tensor_copy(lhs4v[:, :, 0], pid_i[:].to_broadcast([P, TT]))
            nc.vector.tensor_copy(lhs4v[:, :, 1], cw_all[:])
            nc.vector.memset(lhs4v[:, :, 2], 1.0)
            nc.vector.tensor_copy(lhs4v[:, :, 3], tv_i[:])
            psc_a = ps1.tile([4, 512], F32, tag="pd", space="PSUM")
            psc_b = ps1.tile([4, CAP - 512], F32, tag="po", space="PSUM")
            for t in range(TT):
                oh_t = sb.tile([P, CAP], BF16, tag="oh_t", bufs=2)
                nc.vector.tensor_scalar(out=oh_t[:], in0=iotacols[:],
                                        scalar1=slotf[:, t:t + 1], scalar2=None,
                                        op0=OP.is_equal)
                nc.tensor.matmul(psc_a[:], lhsT=lhs4[:, 4 * t:4 * t + 4],
                                 rhs=oh_t[:, 0:512],
                                 start=(t == 0), stop=(t == TT - 1))
                nc.tensor.matmul(psc_b[:], lhsT=lhs4[:, 4 * t:4 * t + 4],
                                 rhs=oh_t[:, 512:CAP],
                                 start=(t == 0), stop=(t == TT - 1))
            csb = sb.tile([4, CAP], F32, tag="csb", bufs=1)
            nc.scalar.copy(csb[:, 0:512], psc_a[:])
            nc.scalar.copy(csb[:, 512:CAP], psc_b[:])
            idx_i = res.tile([P, G], I32)
            cw_slots = res.tile([P, G], F32)
            for g in range(G):
                tpc = ps.tile([P, 4], F32, tag="pB", space="PSUM")
                nc.tensor.transpose(out=tpc[:], in_=csb[:, g * P:(g + 1) * P],
                                    identity=idf[0:4, 0:4])
                scr = sb.tile([P, 4], F32, tag="scr")
                nc.scalar.copy(scr[:], tpc[:])
                idxf = sb.tile([P, 1], F32, tag="idxf")
                nc.vector.scalar_tensor_tensor(out=idxf[:], in0=scr[:, 3:4],
                                               scalar=128.0, in1=scr[:, 0:1],
                                               op0=OP.mult, op1=OP.add)
                emp = sb.tile([P, 1], F32, tag="emp")
                nc.vector.tensor_scalar(out=emp[:], in0=scr[:, 2:3],
                                        scalar1=-2048.0, scalar2=2048.0,
                                        op0=OP.mult, op1=OP.add)
                nc.vector.tensor_add(out=idxf[:], in0=idxf[:], in1=emp[:])
                nc.vector.tensor_copy(idx_i[:, g:g + 1], idxf[:])
                nc.vector.tensor_copy(cw_slots[:, g:g + 1], scr[:, 1:2])

            # ---------------- M6: gather + transpose ----------------
            xgT = [res.tile([P, CAP], BF16, tag=f"xgT{k}", name=f"xgT{k}") for k in range(KH)]
            for g in range(G):
                gx = sb.tile([P, H], BF16, tag="gx")
                nc.vector.memset(gx[:], 0.0)
                nc.gpsimd.indirect_dma_start(
                    out=gx[:], out_offset=None, in_=xnorm_d[:, :],
                    in_offset=bass.IndirectOffsetOnAxis(ap=idx_i[:, g:g + 1], axis=0),
                    bounds_check=T - 1, oob_is_err=False)
                for k in range(KH):
                    ps_tp = ps.tile([P, P], BF16, tag="pB", space="PSUM")
                    nc.tensor.transpose(out=ps_tp[:], in_=gx[:, k * P:(k + 1) * P],
                                        identity=idb[:])
                    nc.scalar.copy(xgT[k][:, g * P:(g + 1) * P], ps_tp[:])

            # Residual-MLP weight preloads (issued interleaved with the FFN
            # streams so they complete before the ReduceScatter hogs the DMA
            # queues; recycled tags keep SBUF flat).
            _ptags = ["wqh", "wqh", "ropeqf",
                      "xsplit", "xsplit", "xsplit", "xsplit", "xsplit",
                      "xn", "xn", "sqs", "sqs", "xns", "xns", "gx", "gx",
                      "srk", "srk", "xkvS", "xkvS", "sqk", "sqk", "lgT", "lgT"]
            _psrc = [(rw1, m) for m in range(KH)] + [(rw3, m) for m in range(KH)] \
                    + [(rw2, m) for m in range(KH)]
            rwpre = []

            def emit_preload():
                i_ = len(rwpre)
                if i_ >= len(_psrc):
                    return
                wsrc, m = _psrc[i_]
                _tg = _ptags[i_]
                _bufs = {"xsplit": 5}.get(_tg, 2)
                tt_ = sb.tile([P, H], BF16, tag=_tg, name=f"rwpre{i_}", bufs=_bufs)
                nc.sync.dma_start(
                    out=tt_[:],
                    in_=wsrc[m, :, :])
                rwpre.append(tt_)

            # ---------------- M7: expert FFN on CAP slots ----------------
            NSC = ((0, 512), (512, CAP - 512))
            hT = [res.tile([P, CAP], BF16, tag=f"hT{m}", name=f"hT{m}") for m in range(FM)]
            for m in range(FM):
                w1m = sb.tile([P, H], BF16, tag="w1m", bufs=2)
                nc.sync.dma_start(
                    out=w1m[:],
                    in_=ew1[m, :, :])
                w3m = sb.tile([P, H], BF16, tag="w3m", bufs=2)
                nc.sync.dma_start(
                    out=w3m[:],
                    in_=ew3[m, :, :])
                p1a = ps.tile([P, 512], F32, tag="pA", space="PSUM", name="p1a")
                p1b = ps.tile([P, 128], F32, tag="pA", space="PSUM", name="p1b")
                p3a = ps.tile([P, 512], F32, tag="pB", space="PSUM", name="p3a")
                p3b = ps.tile([P, 128], F32, tag="pB", space="PSUM", name="p3b")
                for k in range(KH):
                    st, sp = k == 0, k == KH - 1
                    nc.tensor.matmul(p1a[:], lhsT=w1m[:, k * P:(k + 1) * P],
                                     rhs=xgT[k][:, 0:512], start=st, stop=sp)
                    nc.tensor.matmul(p1b[:], lhsT=w1m[:, k * P:(k + 1) * P],
                                     rhs=xgT[k][:, 512:CAP], start=st, stop=sp)
                for k in range(KH):
                    st, sp = k == 0, k == KH - 1
                    nc.tensor.matmul(p3a[:], lhsT=w3m[:, k * P:(k + 1) * P],
                                     rhs=xgT[k][:, 0:512], start=st, stop=sp)
                    nc.tensor.matmul(p3b[:], lhsT=w3m[:, k * P:(k + 1) * P],
                                     rhs=xgT[k][:, 512:CAP], start=st, stop=sp)
                emit_preload()
                for ns0, nsw, p1, p3 in ((0, 512, p1a, p3a), (512, CAP - 512, p1b, p3b)):
                    t1 = sb.tile([P, 512], BF16, tag="t1", name="t1")
                    nc.scalar.activation(out=t1[:, :nsw], in_=p1[:, :nsw], func=AF.Sigmoid)
                    tb = sb.tile([P, 512], BF16, tag="tb", name="tb")
                    nc.vector.tensor_tensor(out=tb[:, :nsw], in0=t1[:, :nsw],
                                            in1=p1[:, :nsw], op=OP.mult)
                    nc.vector.tensor_tensor(out=hT[m][:, ns0:ns0 + nsw],
                                            in0=tb[:, :nsw], in1=p3[:, :nsw], op=OP.mult)
            ynat = [res.tile([P, H], BF16, tag=f"ynat{g}", name=f"ynat{g}") for g in range(G)]
            for mh in range(KH):
                w2a = sb.tile([P, 11 * P], BF16, tag="w2m", bufs=2, name="w2a")
                nc.sync.dma_start(
                    out=w2a[:],
                    in_=ew2[mh, :, 0:11 * P])
                w2b = sb.tile([P, 11 * P], BF16, tag="w2m", bufs=2, name="w2b")
                nc.sync.dma_start(
                    out=w2b[:],
                    in_=ew2[mh, :, 11 * P:F])
                yT = sb.tile([P, CAP], BF16, tag="yT")
                pya = ps.tile([P, 512], F32, tag="pA", space="PSUM", name="pya")
                pyb = ps.tile([P, 128], F32, tag="pB", space="PSUM", name="pyb")
                for k in range(FM):
                    wsrc = w2a if k < 11 else w2b
                    lhs = wsrc[:, (k % 11) * P:(k % 11 + 1) * P]
                    st, sp = k == 0, k == FM - 1
                    nc.tensor.matmul(pya[:], lhsT=lhs, rhs=hT[k][:, 0:512],
                                     start=st, stop=sp)
                    nc.tensor.matmul(pyb[:], lhsT=lhs, rhs=hT[k][:, 512:CAP],
                                     start=st, stop=sp)
                nc.scalar.copy(yT[:, 0:512], pya[:])
                nc.scalar.copy(yT[:, 512:CAP], pyb[:])
                emit_preload()
                for g in range(G):
                    ps_tp = ps.tile([P, P], BF16, tag="pB", space="PSUM")
                    nc.tensor.transpose(out=ps_tp[:], in_=yT[:, g * P:(g + 1) * P],
                                        identity=idb[:])
                    nc.scalar.activation(out=ynat[g][:, mh * P:(mh + 1) * P],
                                         in_=ps_tp[:], func=AF.Copy,
                                         scale=cw_slots[:, g:g + 1])
            for g in range(G):
                nc.gpsimd.indirect_dma_start(
                    out=acc_d[:, :],
                    out_offset=bass.IndirectOffsetOnAxis(ap=idx_i[:, g:g + 1], axis=0),
                    in_=ynat[g][:], in_offset=None,
                    bounds_check=T - 1, oob_is_err=False)

            rw1p, rw3p, rw2p = rwpre[0:KH], rwpre[KH:2 * KH], rwpre[2 * KH:3 * KH]

            # D3: attention per head (all tiles base-partition 0)
            mk = [cpool.tile([P, TS], F32, name=f"mk{c}") for c in range(4)]
            for c in range(4):
                nc.sync.dma_start(out=mk[c][:], in_=maskT[c * P:(c + 1) * P, :])
            attnh = [res.tile([HD, TS], BF16, tag=(f"attnh{h}" if h < 2 else f"qrh{h - 2}"), name=f"attnh{h}") for h in range(NH)]
            for h in range(NH):
                kvh = h // 4
                pd = ps1.tile([P, TS], F32, tag="pd", space="PSUM")
                po = ps1.tile([HD, TS], F32, tag="po", space="PSUM")
                for c in range(4):
                    ps_s = ps.tile([P, TS], F32, tag="pC", space="PSUM")
                    nc.tensor.matmul(ps_s[:],
                                     lhsT=krh[kvh][:, c * P:(c + 1) * P],
                                     rhs=qrh[h][:], start=True, stop=True)
                    sm = sb.tile([P, TS], F32, tag="sm")
                    nc.vector.tensor_add(out=sm[:], in0=ps_s[:], in1=mk[c][:])
                    pT = sb.tile([P, TS], BF16, tag="pT", bufs=3)
                    nc.scalar.activation(out=pT[:], in_=sm[:], func=AF.Exp, scale=0.125)
                    nc.tensor.matmul(pd[:], lhsT=ones_b[:], rhs=pT[:],
                                     start=(c == 0), stop=(c == 3))
                    nc.tensor.matmul(po[:], lhsT=vnat[c][:, kvh * HD:(kvh + 1) * HD],
                                     rhs=pT[:], start=(c == 0), stop=(c == 3))
                rd = sb.tile([HD, TS], F32, tag="rd")
                nc.vector.reciprocal(rd[:], pd[0:HD, :])
                nc.vector.tensor_tensor(out=attnh[h][:], in0=po[:],
                                        in1=rd[:], op=OP.mult)

            # D4: output projection (contraction in 16 chunks of 64) + residual
            RAT = [res.tile([P, TS], F32, tag=f"xgT{m}", name=f"RAT{m}") for m in range(KH)]
            for m in range(KH):
                woa = sb.tile([HD, 8 * P], BF16, tag="wom", bufs=2, name="woa")
                nc.sync.dma_start(
                    out=woa[:],
                    in_=wo[m, :, 0:8 * P])
                wob = sb.tile([HD, 8 * P], BF16, tag="wom", bufs=2, name="wob")
                nc.sync.dma_start(
                    out=wob[:],
                    in_=wo[m, :, 8 * P:NH * P])
                op_ps = ps.tile([P, TS], F32, tag="pB", space="PSUM")
                for k in range(NH):
                    wsrc = woa if k < 8 else wob
                    nc.tensor.matmul(op_ps[:], lhsT=wsrc[:, (k % 8) * P:(k % 8 + 1) * P],
                                     rhs=attnh[k][:], start=(k == 0), stop=(k == NH - 1))
                xres = sb.tile([P, TS], F32, tag="xres", name="xres")
                nc.sync.dma_start(out=xres[:], in_=xT_kv[m * P:(m + 1) * P, TS:KV])
                nc.vector.tensor_add(out=RAT[m][:], in0=op_ps[:], in1=xres[:])

            # D5: residual MLP
            ps_rm = ps.tile([P, TS], F32, tag="pA", space="PSUM")
            for m in range(KH):
                sqm = sb.tile([P, TS], BF16, tag="sqm")
                nc.scalar.activation(out=sqm[:], in_=RAT[m][:], func=AF.Square)
                nc.tensor.matmul(ps_rm[:], lhsT=ones_b[:], rhs=sqm[:],
                                 start=(m == 0), stop=(m == KH - 1))
            srm = sb.tile([P, TS], F32, tag="srm")
            nc.scalar.activation(out=srm[:], in_=ps_rm[:], func=AF.Sqrt,
                                 scale=1.0 / H, bias=epsb[:])
            rrm = sb.tile([P, TS], F32, tag="rrm", bufs=1)
            nc.vector.reciprocal(rrm[:], srm[:])
            xmT = [res.tile([P, TS], BF16, tag=f"hT{16 + m}" if m < 6 else f"ynat{m - 6}", name=f"xmT{m}") for m in range(KH)]
            for m in range(KH):
                nc.vector.tensor_mul(out=xmT[m][:], in0=RAT[m][:], in1=rrm[:])
            hm = [res.tile([P, TS], BF16, tag=f"hT{8 + m}", name=f"hm{m}") for m in range(KH)]
            for m in range(KH):
                p1 = ps.tile([P, TS], F32, tag="pB", space="PSUM")
                for k in range(KH):
                    nc.tensor.matmul(p1[:], lhsT=rw1p[m][:, k * P:(k + 1) * P],
                                     rhs=xmT[k][:], start=(k == 0), stop=(k == KH - 1))
                p3 = ps.tile([P, TS], F32, tag="pC", space="PSUM")
                for k in range(KH):
                    nc.tensor.matmul(p3[:], lhsT=rw3p[m][:, k * P:(k + 1) * P],
                                     rhs=xmT[k][:], start=(k == 0), stop=(k == KH - 1))
                t1 = sb.tile([P, TS], BF16, tag="t1d")
                nc.scalar.activation(out=t1[:], in_=p1[:], func=AF.Sigmoid)
                tb = sb.tile([P, TS], BF16, tag="tbd")
                nc.vector.tensor_tensor(out=tb[:], in0=t1[:], in1=p1[:], op=OP.mult)
                nc.vector.tensor_tensor(out=hm[m][:], in0=tb[:], in1=p3[:], op=OP.mult)

            # D6a: rw2 + residual accumulated in place into RAT (pre-collective)
            for m in range(KH):
                p2 = ps.tile([P, TS], F32, tag="pB", space="PSUM")
                for k in range(KH):
                    nc.tensor.matmul(p2[:], lhsT=rw2p[m][:, k * P:(k + 1) * P],
                                     rhs=hm[k][:], start=(k == 0), stop=(k == KH - 1))
                nc.vector.tensor_add(out=RAT[m][:], in0=p2[:], in1=RAT[m][:])
            # ---------------- M8: ReduceScatter ----------------
            nc.gpsimd.collective_compute(
                "ReduceScatter", OP.add, replica_groups=[list(range(NCORES))],
                ins=[acc_d.ap().opt()], outs=[rs_d.ap().opt()])

            # D6b: MoE slice transpose + final sum -> output
            moeT = [res.tile([P, TS], F32, tag=f"hT{k}", name=f"moeT{k}") for k in range(KH)]
            for pt in range(2):
                rsb = sb.tile([P, H], BF16, tag="rsb")
                nc.sync.dma_start(out=rsb[:], in_=rs_d[pt * P:(pt + 1) * P, :])
                for k in range(KH):
                    ps_tp = ps.tile([P, P], BF16, tag="pB", space="PSUM")
                    nc.tensor.transpose(out=ps_tp[:], in_=rsb[:, k * P:(k + 1) * P],
                                        identity=idb[:])
                    nc.scalar.copy(moeT[k][:, pt * P:(pt + 1) * P], ps_tp[:])
            for m in range(KH):
                ot = sb.tile([P, TS], F32, tag="ot")
                nc.vector.tensor_add(out=ot[:], in0=RAT[m][:], in1=moeT[m][:])
                nc.sync.dma_start(out=out[m * P:(m + 1) * P, :], in_=ot[:])

    nc.finalize()
    _BUILD_CACHE["nc"] = nc
    return nc


def _host_prep(inputs):
    f32 = np.float32
    x = np.asarray(inputs["hidden_states"], f32).reshape(T, H)
    ln1 = np.asarray(inputs["ln1_w"], f32)
    res_ln = np.asarray(inputs["res_ln_w"], f32)
    post_ln = np.asarray(inputs["post_ln_w"], f32)

    import ml_dtypes
    bf16 = ml_dtypes.bfloat16

    def b(a):
        return np.ascontiguousarray(np.asarray(a, f32)).astype(bf16)

    def mmaj(w, pp, mm):
        # [K, M] -> [M//mm, pp, (K//pp)*mm] with w[k, m] at [m//mm, k%pp, (k//pp)*mm + m%mm]
        K, M = w.shape
        return np.ascontiguousarray(
            w.reshape(K // pp, pp, M // mm, mm).transpose(2, 1, 0, 3).reshape(M // mm, pp, (K // pp) * mm))

    wq = mmaj(b(ln1[:, None] * np.asarray(inputs["q_w"], f32)), 128, 64)
    wk = mmaj(b(ln1[:, None] * np.asarray(inputs["k_w"], f32)), 128, 64)
    wv = mmaj(b(ln1[:, None] * np.asarray(inputs["v_w"], f32)), 128, 128)
    wo = mmaj(b(inputs["o_w"]), 64, 128)
    rw1 = mmaj(b(res_ln[:, None] * np.asarray(inputs["rw1"], f32)), 128, 128)
    rw3 = mmaj(b(res_ln[:, None] * np.asarray(inputs["rw3"], f32)), 128, 128)
    rw2 = mmaj(b(inputs["rw2"]), 128, 128)
    gate = np.ascontiguousarray(post_ln[:, None] * np.asarray(inputs["gate_w"], f32))
    gatep = np.ascontiguousarray(gate.reshape(8, 128, 8).transpose(1, 0, 2).reshape(128, 64))
    xT = np.ascontiguousarray(x.T)                       # [H, T]

    e_w1 = np.asarray(inputs["e_w1"], f32)
    e_w3 = np.asarray(inputs["e_w3"], f32)
    e_w2 = np.asarray(inputs["e_w2"], f32)

    # RoPE tables: cos64[d, pos] with d in [0,64), duplicated inv-freq halves
    pos = np.arange(S, dtype=f32)
    inv = 1.0 / (THETA ** (np.arange(0, HD, 2, dtype=f32) / HD))   # [32]
    ang = inv[:, None] * pos[None, :]                               # [32, S]
    cos64 = np.concatenate([np.cos(ang)] * 2, 0)                    # [64, S]
    sin64 = np.concatenate([np.sin(ang)] * 2, 0)

    in_maps = []
    for core in range(NCORES):
        bi, c = divmod(core, 4)
        lo = bi * S + c * TS
        # kv window: previous chunk + own chunk (zeros for c == 0)
        xkv = np.zeros((H, KV), f32)
        if c > 0:
            xkv[:, :TS] = xT[:, lo - TS:lo]
        xkv[:, TS:] = xT[:, lo:lo + TS]
        # mask: valid iff ql < kl <= ql + TS (and kl >= TS when c == 0)
        ql = np.arange(TS)[None, :]
        kl = np.arange(KV)[:, None]
        valid = (kl > ql) & (kl <= ql + TS)
        if c == 0:
            valid &= kl >= TS
        maskT = np.where(valid, 0.0, NEG).astype(f32)
        # RoPE positions (within-sequence)
        pq = c * TS + np.arange(TS)
        pk = np.clip((c - 1) * TS + np.arange(KV), 0, S - 1)
        cq = np.tile(cos64[:, pq], (2, 1)).astype(f32)
        sqv = np.tile(sin64[:, pq], (2, 1)).astype(f32)
        ckv = np.tile(cos64[:, pk], (2, 1)).astype(f32)
        skv = np.tile(sin64[:, pk], (2, 1)).astype(f32)
        oh = np.zeros((P, E), f32)
        oh[:, core] = 1.0
        in_maps.append(dict(
            xT_kv=xkv, xnat=x.astype(bf16), xT=xT, gatep=gatep, onehot=oh,
            cos_q=cq, sin_q=sqv, cos_k=ckv, sin_k=skv, maskT=maskT,
            wq=wq, wk=wk, wv=wv, wo=wo, rw1=rw1, rw3=rw3, rw2=rw2,
            ew1=mmaj(b(post_ln[:, None] * e_w1[core]), 128, 128),
            ew3=mmaj(b(post_ln[:, None] * e_w3[core]), 128, 128),
            ew2=mmaj(b(e_w2[core]), 128, 128),
        ))
    return in_maps


def kernel(**inputs) -> np.ndarray:
    nc = _build()
    in_maps = _host_prep(inputs)
    res = run_bass_kernel_spmd(nc, in_maps, core_ids=list(range(NCORES)))
    outs = [np.asarray(res.results[i]["out"], np.float32).T for i in range(NCORES)]
    full = np.concatenate(outs, 0)          # [T, H] in core order == token order
    return full.reshape(B, S, H)


# revision 38
# speedup vs baseline: 1.1825x; 1.0613x over previous
"""Arctic decoder layer (attention + residual MLP + top-2 MoE) on 8 TRN2 NeuronCores.

Strategy:
  - Data parallel over tokens for attention/norms/residual MLP (256 tokens/core,
    sliding-window attention needs only the previous 256-token chunk as halo).
  - Expert parallel for the MoE: every core receives the full (replicated) input,
    computes gating for all 2048 tokens, compacts the token indices routed to ITS
    expert (capacity 640), gathers them with indirect DMA, runs the expert FFN on
    the gathered tokens only, scales by the combine weights and scatters into a
    zeroed [2048, 1024] accumulator; one ReduceScatter(add) returns each core its
    own 256-token slice of the MoE output.
  - Activations live as [feature, token] (transposed) for matmuls; natural
    [token, feature] layout is used for RMS statistics, gating softmax/top-2 and
    the gather/scatter.  Matmuls run in bf16 (f32 PSUM accumulation); the gating
    logits use a bf16 split-float (hi+lo) product to keep top-2 selection exact.
"""
import os
import sys

for _p in ("/opt/trn_rl_repo", "/root/.axon_site/_ro/trn_rl_repo", "/root/.axon_site"):
    if os.path.isdir(_p) and _p not in sys.path:
        sys.path.append(_p)

import numpy as np

import concourse.bass as bass
import concourse.bacc as bacc
import concourse.mybir as mybir
import concourse.tile as tile
from concourse.bass_utils import run_bass_kernel_spmd
from concourse.masks import make_identity

F32 = mybir.dt.float32
BF16 = mybir.dt.bfloat16
I32 = mybir.dt.int32
AF = mybir.ActivationFunctionType
OP = mybir.AluOpType
AX = mybir.AxisListType

NCORES = 8
P = 128
B, S, H = 2, 1024, 1024
T = B * S                 # 2048 tokens
TT = T // P               # 16 token tiles
KH = H // P               # 8 hidden k-chunks
NH, NKV, HD = 16, 4, 64
F = 2816
FM = F // P               # 22
E = 8
CAP = 640                 # per-expert token capacity (actual max load is ~531)
G = CAP // P              # 5 slot batches
TS = T // NCORES          # 256 tokens per core
KV = 2 * TS               # 512 kv-window tokens per core
EPS = 1e-5
THETA = 10000.0
NEG = -1.0e5              # additive mask value (pre-exp)

_BUILD_CACHE = {}


def _build():
    if "nc" in _BUILD_CACHE:
        return _BUILD_CACHE["nc"]
    nc = bacc.Bacc("TRN2", target_bir_lowering=False, debug=False, num_devices=NCORES)

    dp = nc.declare_dram_parameter
    xT_kv = dp("xT_kv", [H, KV], F32, isOutput=False)
    xnat = dp("xnat", [T, H], BF16, isOutput=False)
    xT = dp("xT", [H, T], F32, isOutput=False)
    onehot = dp("onehot", [P, E], F32, isOutput=False)
    cos_q = dp("cos_q", [P, TS], F32, isOutput=False)
    sin_q = dp("sin_q", [P, TS], F32, isOutput=False)
    cos_k = dp("cos_k", [P, KV], F32, isOutput=False)
    sin_k = dp("sin_k", [P, KV], F32, isOutput=False)
    maskT = dp("maskT", [KV, TS], F32, isOutput=False)
    wq = dp("wq", [NH, P, KH * HD], BF16, isOutput=False)
    wk = dp("wk", [NKV, P, KH * HD], BF16, isOutput=False)
    wv = dp("wv", [2, P, H], BF16, isOutput=False)
    wo = dp("wo", [KH, HD, NH * P], BF16, isOutput=False)
    rw1 = dp("rw1", [KH, P, H], BF16, isOutput=False)
    rw3 = dp("rw3", [KH, P, H], BF16, isOutput=False)
    rw2 = dp("rw2", [KH, P, H], BF16, isOutput=False)
    ew1 = dp("ew1", [FM, P, H], BF16, isOutput=False)
    ew3 = dp("ew3", [FM, P, H], BF16, isOutput=False)
    ew2 = dp("ew2", [KH, P, F], BF16, isOutput=False)
    gatep = dp("gatep", [P, KH * E], F32, isOutput=False)
    out = dp("out", [H, TS], F32, isOutput=True)

    # internal DRAM (offset-0 targets for indirect DMA + collective bounces)
    xnorm_d = nc.dram_tensor("xnorm_d", [T, H], BF16)
    acc_d = nc.dram_tensor("acc_d", [T, H], BF16)
    rs_d = nc.dram_tensor("rs_d", [TS, H], BF16)

    with tile.TileContext(nc) as tc:
        with (
            tc.tile_pool(name="const", bufs=1) as cpool,
            tc.tile_pool(name="sb", bufs=2) as sb,
            tc.tile_pool(name="res", bufs=1) as res,
            tc.tile_pool(name="ps", bufs=2, space="PSUM") as ps,
            tc.tile_pool(name="ps1", bufs=1, space="PSUM") as ps1,
        ):
            # ---------------- constants ----------------
            idf = cpool.tile([P, P], F32)
            make_identity(nc, idf[:])
            idb = cpool.tile([P, P], BF16)
            make_identity(nc, idb[:])
            ones_b = cpool.tile([P, P], BF16)
            nc.vector.memset(ones_b[:], 1.0)
            # strict lower-triangular LT[k, m] = 1 if k < m (for exclusive cumsum)
            lt128 = cpool.tile([P, P], F32)
            nc.gpsimd.memset(lt128[:], 0.0)
            nc.gpsimd.affine_select(out=lt128[:], in_=lt128[:], pattern=[[-1, P]],
                                    compare_op=OP.is_ge, fill=1.0, base=0,
                                    channel_multiplier=1)
            lt16 = cpool.tile([TT, TT], F32)
            nc.gpsimd.memset(lt16[:], 0.0)
            nc.gpsimd.affine_select(out=lt16[:], in_=lt16[:], pattern=[[-1, TT]],
                                    compare_op=OP.is_ge, fill=1.0, base=0,
                                    channel_multiplier=1)
            # signed rotate-half permutation for RoPE: rot[m] = -q[m+32] | q[m-32]
            r64 = np.zeros((HD, HD), np.float32)
            for mm in range(32):
                r64[mm + 32, mm] = -1.0
                r64[mm, mm + 32] = 1.0
            import ml_dtypes as _mld
            r64_d = nc.inline_tensor(r64.astype(_mld.bfloat16), name="r64_const")
            r64t = cpool.tile([HD, HD], BF16)
            nc.sync.dma_start(out=r64t[:], in_=r64_d[:, :])
            epsb = cpool.tile([P, 1], F32)
            nc.vector.memset(epsb[:], EPS)
            zb = cpool.tile([P, H], BF16)
            nc.vector.memset(zb[:], 0.0)
            for t in range(TT):
                nc.sync.dma_start(out=acc_d[t * P:(t + 1) * P, :], in_=zb[:])

            # ================= DP path (overlaps the collective) =============
            # D1: RMS over the 512-token kv window (transposed layout)
            ps_rms = ps.tile([P, KV], F32, tag="pA", space="PSUM")
            for k in range(KH):
                xk1 = sb.tile([P, KV], F32, tag="xkvS", name="xk1")
                nc.sync.dma_start(out=xk1[:], in_=xT_kv[k * P:(k + 1) * P, :])
                sqk = sb.tile([P, KV], BF16, tag="sqk")
                nc.scalar.activation(out=sqk[:], in_=xk1[:], func=AF.Square)
                nc.tensor.matmul(ps_rms[:], lhsT=ones_b[:], rhs=sqk[:],
                                 start=(k == 0), stop=(k == KH - 1))
            srk = sb.tile([P, KV], F32, tag="srk")
            nc.scalar.activation(out=srk[:], in_=ps_rms[:], func=AF.Sqrt,
                                 scale=1.0 / H, bias=epsb[:])
            rkv = sb.tile([P, KV], F32, tag="rkv", bufs=1)
            nc.vector.reciprocal(rkv[:], srk[:])
            xnkv = [res.tile([P, KV], BF16, tag=f"xnkv{k}", name=f"xnkv{k}") for k in range(KH)]
            for k in range(KH):
                xk2 = sb.tile([P, KV], F32, tag="xkvS", name="xk2")
                nc.sync.dma_start(out=xk2[:], in_=xT_kv[k * P:(k + 1) * P, :])
                nc.vector.tensor_mul(out=xnkv[k][:], in0=xk2[:], in1=rkv[:])

            # D2: q/k/v projections + RoPE + v transpose
            cq = cpool.tile([P, TS], F32)
            nc.sync.dma_start(out=cq[:], in_=cos_q[:, :])
            sq = cpool.tile([P, TS], F32)
            nc.sync.dma_start(out=sq[:], in_=sin_q[:, :])
            ck = cpool.tile([P, KV], F32)
            nc.sync.dma_start(out=ck[:], in_=cos_k[:, :])
            sk = cpool.tile([P, KV], F32)
            nc.sync.dma_start(out=sk[:], in_=sin_k[:, :])

            def rope(src_ps, cos_t, sin_t, w, dst, tagsfx):
                # src_ps: [HD, w] psum f32 (one head); dst: [HD, w] bf16 sbuf
                qf = sb.tile([HD, KV], BF16, tag="ropeqf", name="ropeqf")
                nc.scalar.copy(qf[:, :w], src_ps[:, :w])
                rot = ps.tile([HD, KV], F32, tag="pC", space="PSUM", name="roperot")
                nc.tensor.matmul(rot[:, :w], lhsT=r64t[:], rhs=qf[:, :w],
                                 start=True, stop=True)
                t1 = sb.tile([HD, KV], F32, tag="ropet1", name="ropet1")
                nc.vector.tensor_mul(out=t1[:, :w], in0=qf[:, :w], in1=cos_t[0:HD, :w])
                nc.vector.tensor_mul(out=dst, in0=rot[:, :w], in1=sin_t[0:HD, :w])
                nc.vector.tensor_add(out=dst, in0=t1[:, :w], in1=dst)

            # per-head q (16 x [64, 256]) and per-kv-head k (4 x [64, 512])
            qrh = [res.tile([HD, TS], BF16, tag=f"qrh{h}", name=f"qrh{h}") for h in range(NH)]
            for h in range(NH):
                wqh = sb.tile([P, KH * HD], BF16, tag="wqh")
                nc.sync.dma_start(
                    out=wqh[:],
                    in_=wq[h, :, :])
                qp = ps.tile([HD, TS], F32, tag="pB", space="PSUM")
                for k in range(KH):
                    nc.tensor.matmul(qp[:], lhsT=wqh[:, k * HD:(k + 1) * HD],
                                     rhs=xnkv[k][:, TS:KV],
                                     start=(k == 0), stop=(k == KH - 1))
                rope(qp, cq, sq, TS, qrh[h][:], "q")
            krh = [res.tile([HD, KV], BF16, tag=f"krh{h}", name=f"krh{h}") for h in range(NKV)]
            vnat = [res.tile([P, NKV * HD], BF16, tag=f"vnat{c}", name=f"vnat{c}") for c in range(4)]
            for h in range(NKV):
                wkh = sb.tile([P, KH * HD], BF16, tag="wqh")
                nc.sync.dma_start(
                    out=wkh[:],
                    in_=wk[h, :, :])
                kp = ps.tile([HD, KV], F32, tag="pA", space="PSUM")
                for k in range(KH):
                    nc.tensor.matmul(kp[:], lhsT=wkh[:, k * HD:(k + 1) * HD],
                                     rhs=xnkv[k][:],
                                     start=(k == 0), stop=(k == KH - 1))
                rope(kp, ck, sk, KV, krh[h][:], "k")
            for m in range(2):
                wvm = sb.tile([P, H], BF16, tag="wqh")
                nc.sync.dma_start(
                    out=wvm[:],
                    in_=wv[m, :, :])
                vp = ps.tile([P, KV], F32, tag="pA", space="PSUM")
                for k in range(KH):
                    nc.tensor.matmul(vp[:], lhsT=wvm[:, k * P:(k + 1) * P],
                                     rhs=xnkv[k][:],
                                     start=(k == 0), stop=(k == KH - 1))
                vT = sb.tile([P, KV], BF16, tag="vT")
                nc.scalar.copy(vT[:], vp[:])
                for c in range(4):
                    ps_tp = ps.tile([P, P], BF16, tag="pB", space="PSUM")
                    nc.tensor.transpose(out=ps_tp[:], in_=vT[:, c * P:(c + 1) * P],
                                        identity=idb[:])
                    nc.scalar.copy(vnat[c][:, m * P:(m + 1) * P], ps_tp[:])


            # ---------------- M1: natural RMS over all tokens ----------------
            rinv_all = res.tile([P, TT], F32)
            for t in range(TT):
                xn = sb.tile([P, H], BF16, tag="xn")
                nc.sync.dma_start(out=xn[:], in_=xnat[t * P:(t + 1) * P, :])
                sqs = sb.tile([P, H], BF16, tag="sqs")
                ssq = sb.tile([P, 1], F32, tag="ssq")
                nc.scalar.activation(out=sqs[:], in_=xn[:], func=AF.Square,
                                     accum_out=ssq[:])
                srt = sb.tile([P, 1], F32, tag="srt")
                nc.scalar.activation(out=srt[:], in_=ssq[:], func=AF.Sqrt,
                                     scale=1.0 / H, bias=epsb[:])
                nc.vector.reciprocal(rinv_all[:, t:t + 1], srt[:])
                xns = sb.tile([P, H], BF16, tag="xns")
                nc.scalar.activation(out=xns[:], in_=xn[:], func=AF.Copy,
                                     scale=rinv_all[:, t:t + 1])
                nc.sync.dma_start(out=xnorm_d[t * P:(t + 1) * P, :], in_=xns[:])

            # ---------------- M2+M3+M4: gating ----------------
            gs = cpool.tile([P, KH * E], F32)
            nc.sync.dma_start(out=gs[:], in_=gatep[:, :])
            oh = cpool.tile([P, E], F32)
            nc.sync.dma_start(out=oh[:], in_=onehot[:, :])

            cw_all = res.tile([P, TT], F32)
            mask_all = res.tile([P, TT], F32)
            for n in range(T // 512):
                ps_lg = ps.tile([E, 512], F32, tag="pA", space="PSUM")
                for k in range(KH):
                    xs = sb.tile([P, 512], F32, tag="xsplit", bufs=5)
                    nc.sync.dma_start(
                        out=xs[:], in_=xT[k * P:(k + 1) * P, n * 512:(n + 1) * 512])
                    nc.tensor.matmul(
                        ps_lg[:], lhsT=gs[:, k * E:(k + 1) * E], rhs=xs[:],
                        start=(k == 0), stop=(k == KH - 1))
                lgT = sb.tile([E, 512], F32, tag="lgT")
                nc.scalar.copy(lgT[:], ps_lg[:])
                for j in range(4):
                    t = n * 4 + j
                    ps_tp = ps.tile([P, E], F32, tag="pB", space="PSUM")
                    nc.tensor.transpose(out=ps_tp[:], in_=lgT[:, j * P:(j + 1) * P],
                                        identity=idf[0:E, 0:E])
                    lg = sb.tile([P, E], F32, tag="lg")
                    nc.scalar.activation(out=lg[:], in_=ps_tp[:], func=AF.Copy,
                                         scale=rinv_all[:, t:t + 1])
                    # softmax + top2
                    ngm = sb.tile([P, 1], F32, tag="ngm")
                    nc.vector.tensor_reduce(out=ngm[:], in_=lg[:], axis=AX.X,
                                            op=OP.max, negate=True)
                    probs = sb.tile([P, E], F32, tag="probs")
                    nc.scalar.activation(out=probs[:], in_=lg[:], func=AF.Exp,
                                         bias=ngm[:])
                    top8 = sb.tile([P, E], F32, tag="top8")
                    nc.vector.max(out=top8[:], in_=probs[:])
                    den = sb.tile([P, 1], F32, tag="den")
                    nc.vector.tensor_add(out=den[:], in0=top8[:, 0:1], in1=top8[:, 1:2])
                    rden = sb.tile([P, 1], F32, tag="rden")
                    nc.vector.reciprocal(rden[:], den[:])
                    pex = sb.tile([P, E], F32, tag="pex")
                    nc.vector.tensor_mul(out=pex[:], in0=probs[:], in1=oh[:])
                    pe = sb.tile([P, 1], F32, tag="pe")
                    nc.vector.reduce_sum(out=pe[:], in_=pex[:], axis=AX.X)
                    nc.vector.tensor_tensor(out=mask_all[:, t:t + 1], in0=pe[:],
                                            in1=top8[:, 1:2], op=OP.is_ge)
                    cw0 = sb.tile([P, 1], F32, tag="cw0")
                    nc.vector.tensor_mul(out=cw0[:], in0=pe[:], in1=mask_all[:, t:t + 1])
                    nc.vector.tensor_mul(out=cw_all[:, t:t + 1], in0=cw0[:], in1=rden[:])

            # ---------------- M5: compaction ----------------
            ps_mt = ps.tile([TT, P], F32, tag="pB", space="PSUM")
            nc.tensor.transpose(out=ps_mt[:], in_=mask_all[:], identity=idf[:])
            mtp = sb.tile([TT, P], F32, tag="mtp")
            nc.scalar.copy(mtp[:], ps_mt[:])
            cs = sb.tile([TT, 1], F32, tag="cs")
            nc.vector.reduce_sum(out=cs[:], in_=mtp[:], axis=AX.X)
            ps_pos = ps.tile([P, TT], F32, tag="pA", space="PSUM")
            nc.tensor.matmul(ps_pos[:], lhsT=lt128[:], rhs=mask_all[:],
                             start=True, stop=False)
            nc.tensor.matmul(ps_pos[:], lhsT=cs[:].to_broadcast([TT, P]),
                             rhs=lt16[:], start=False, stop=True)
            slotf = sb.tile([P, TT], F32, tag="slotf")
            nc.vector.scalar_tensor_tensor(out=slotf[:], in0=ps_pos[:], scalar=4096.0,
                                           in1=mask_all[:], op0=OP.subtract, op1=OP.mult)
            nc.vector.tensor_scalar_add(slotf[:], slotf[:], 4096.0)
            # one-hot compaction: psc rows = [sum pid*oh, sum cw*oh, occ, sum t*oh]
            pid_i = sb.tile([P, 1], I32, tag="pid_i")
            nc.gpsimd.iota(pid_i[:], pattern=[[0, 1]], base=0, channel_multiplier=1)
            tv_i = sb.tile([P, TT], I32, tag="tv_i")
            nc.gpsimd.iota(tv_i[:], pattern=[[1, TT]], base=0, channel_multiplier=0)
            ic_scr = sb.tile([P, CAP], I32, tag="csb", bufs=1)
            nc.gpsimd.iota(ic_scr[:], pattern=[[1, CAP]], base=0, channel_multiplier=0)
            iotacols = cpool.tile([P, CAP], F32)
            nc.vector.tensor_copy(iotacols[:], ic_scr[:])
            lhs4 = cpool.tile([P, 4 * TT], BF16)
            lhs4v = lhs4.rearrange("p (t four) -> p t four", four=4)
            nc.vector.tensor_copy(lhs4v[:, :, 0], pid_i[:].to_broadcast([P, TT]))
            nc.vector.tensor_copy(lhs4v[:, :, 1], cw_all[:])
            nc.vector.memset(lhs4v[:, :, 2], 1.0)
            nc.vector.tensor_copy(lhs4v[:, :, 3], tv_i[:])
            psc_a = ps1.tile([4, 512], F32, tag="pd", space="PSUM")
            psc_b = ps1.tile([4, CAP - 512], F32, tag="po", space="PSUM")
            for t in range(TT):
                oh_t = sb.tile([P, CAP], BF16, tag="oh_t", bufs=2)
                nc.vector.tensor_scalar(out=oh_t[:], in0=iotacols[:],
                                        scalar1=slotf[:, t:t + 1], scalar2=None,
                                        op0=OP.is_equal)
                nc.tensor.matmul(psc_a[:], lhsT=lhs4[:, 4 * t:4 * t + 4],
                                 rhs=oh_t[:, 0:512],
                                 start=(t == 0), stop=(t == TT - 1))
                nc.tensor.matmul(psc_b[:], lhsT=lhs4[:, 4 * t:4 * t + 4],
                                 rhs=oh_t[:, 512:CAP],
                                 start=(t == 0), stop=(t == TT - 1))
            csb = sb.tile([4, CAP], F32, tag="csb", bufs=1)
            nc.scalar.copy(csb[:, 0:512], psc_a[:])
            nc.scalar.copy(csb[:, 512:CAP], psc_b[:])
            idx_i = res.tile([P, G], I32)
            cw_slots = res.tile([P, G], F32)
            for g in range(G):
                tpc = ps.tile([P, 4], F32, tag="pB", space="PSUM")
                nc.tensor.transpose(out=tpc[:], in_=csb[:, g * P:(g + 1) * P],
                                    identity=idf[0:4, 0:4])
                scr = sb.tile([P, 4], F32, tag="scr")
                nc.scalar.copy(scr[:], tpc[:])
                idxf = sb.tile([P, 1], F32, tag="idxf")
                nc.vector.scalar_tensor_tensor(out=idxf[:], in0=scr[:, 3:4],
                                               scalar=128.0, in1=scr[:, 0:1],
                                               op0=OP.mult, op1=OP.add)
                emp = sb.tile([P, 1], F32, tag="emp")
                nc.vector.tensor_scalar(out=emp[:], in0=scr[:, 2:3],
                                        scalar1=-2048.0, scalar2=2048.0,
                                        op0=OP.mult, op1=OP.add)
                nc.vector.tensor_add(out=idxf[:], in0=idxf[:], in1=emp[:])
                nc.vector.tensor_copy(idx_i[:, g:g + 1], idxf[:])
                nc.vector.tensor_copy(cw_slots[:, g:g + 1], scr[:, 1:2])

            # ---------------- M6: gather + transpose ----------------
            xgT = [res.tile([P, CAP], BF16, tag=f"xgT{k}", name=f"xgT{k}") for k in range(KH)]
            for g in range(G):
                gx = sb.tile([P, H], BF16, tag="gx")
                nc.vector.memset(gx[:], 0.0)
                nc.gpsimd.indirect_dma_start(
                    out=gx[:], out_offset=None, in_=xnorm_d[:, :],
                    in_offset=bass.IndirectOffsetOnAxis(ap=idx_i[:, g:g + 1], axis=0),
                    bounds_check=T - 1, oob_is_err=False)
                for k in range(KH):
                    ps_tp = ps.tile([P, P], BF16, tag="pB", space="PSUM")
                    nc.tensor.transpose(out=ps_tp[:], in_=gx[:, k * P:(k + 1) * P],
                                        identity=idb[:])
                    nc.scalar.copy(xgT[k][:, g * P:(g + 1) * P], ps_tp[:])

            # Residual-MLP weight preloads (issued interleaved with the FFN
            # streams so they complete before the ReduceScatter hogs the DMA
            # queues; recycled tags keep SBUF flat).
            _ptags = ["wqh", "wqh", "csb",
                      "xsplit", "xsplit", "xsplit", "xsplit", "xsplit",
                      "xn", "xn", "sqs", "sqs", "xns", "xns", "gx", "gx",
                      "srk", "srk", "xkvS", "xkvS", "sqk", "sqk", "lgT", "lgT"]
            _psrc = [(rw1, m) for m in range(KH)] + [(rw3, m) for m in range(KH)] \
                    + [(rw2, m) for m in range(KH)]
            rwpre = []

            def emit_preload():
                i_ = len(rwpre)
                if i_ >= len(_psrc):
                    return
                wsrc, m = _psrc[i_]
                _tg = _ptags[i_]
                _bufs = {"xsplit": 5, "csb": 1}.get(_tg, 2)
                tt_ = sb.tile([P, H], BF16, tag=_tg, name=f"rwpre{i_}", bufs=_bufs)
                nc.sync.dma_start(
                    out=tt_[:],
                    in_=wsrc[m, :, :])
                rwpre.append(tt_)

            # ---------------- M7: expert FFN on CAP slots ----------------
            NSC = ((0, 512), (512, CAP - 512))
            hT = [res.tile([P, CAP], BF16, tag=f"hT{m}", name=f"hT{m}") for m in range(FM)]
            for m in range(FM):
                w1m = sb.tile([P, H], BF16, tag="w1m", bufs=2)
                nc.sync.dma_start(
                    out=w1m[:],
                    in_=ew1[m, :, :])
                w3m = sb.tile([P, H], BF16, tag="w3m", bufs=2)
                nc.sync.dma_start(
                    out=w3m[:],
                    in_=ew3[m, :, :])
                p1a = ps.tile([P, 512], F32, tag="pA", space="PSUM", name="p1a")
                p1b = ps.tile([P, 128], F32, tag="pA", space="PSUM", name="p1b")
                p3a = ps.tile([P, 512], F32, tag="pB", space="PSUM", name="p3a")
                p3b = ps.tile([P, 128], F32, tag="pB", space="PSUM", name="p3b")
                for k in range(KH):
                    st, sp = k == 0, k == KH - 1
                    nc.tensor.matmul(p1a[:], lhsT=w1m[:, k * P:(k + 1) * P],
                                     rhs=xgT[k][:, 0:512], start=st, stop=sp)
                    nc.tensor.matmul(p1b[:], lhsT=w1m[:, k * P:(k + 1) * P],
                                     rhs=xgT[k][:, 512:CAP], start=st, stop=sp)
                for k in range(KH):
                    st, sp = k == 0, k == KH - 1
                    nc.tensor.matmul(p3a[:], lhsT=w3m[:, k * P:(k + 1) * P],
                                     rhs=xgT[k][:, 0:512], start=st, stop=sp)
                    nc.tensor.matmul(p3b[:], lhsT=w3m[:, k * P:(k + 1) * P],
                                     rhs=xgT[k][:, 512:CAP], start=st, stop=sp)
                emit_preload()
                for ns0, nsw, p1, p3 in ((0, 512, p1a, p3a), (512, CAP - 512, p1b, p3b)):
                    t1 = sb.tile([P, 512], BF16, tag="t1", name="t1")
                    nc.scalar.activation(out=t1[:, :nsw], in_=p1[:, :nsw], func=AF.Sigmoid)
                    tb = sb.tile([P, 512], BF16, tag="tb", name="tb")
                    nc.vector.tensor_tensor(out=tb[:, :nsw], in0=t1[:, :nsw],
                                            in1=p1[:, :nsw], op=OP.mult)
                    nc.vector.tensor_tensor(out=hT[m][:, ns0:ns0 + nsw],
                                            in0=tb[:, :nsw], in1=p3[:, :nsw], op=OP.mult)
            ynat = [res.tile([P, H], BF16, tag=f"ynat{g}", name=f"ynat{g}") for g in range(G)]
            for mh in range(KH):
                w2a = sb.tile([P, 11 * P], BF16, tag="w2m", bufs=2, name="w2a")
                nc.sync.dma_start(
                    out=w2a[:],
                    in_=ew2[mh, :, 0:11 * P])
                w2b = sb.tile([P, 11 * P], BF16, tag="w2m", bufs=2, name="w2b")
                nc.sync.dma_start(
                    out=w2b[:],
                    in_=ew2[mh, :, 11 * P:F])
                yT = sb.tile([P, CAP], BF16, tag="yT")
                pya = ps.tile([P, 512], F32, tag="pA", space="PSUM", name="pya")
                pyb = ps.tile([P, 128], F32, tag="pB", space="PSUM", name="pyb")
                for k in range(FM):
                    wsrc = w2a if k < 11 else w2b
                    lhs = wsrc[:, (k % 11) * P:(k % 11 + 1) * P]
                    st, sp = k == 0, k == FM - 1
                    nc.tensor.matmul(pya[:], lhsT=lhs, rhs=hT[k][:, 0:512],
                                     start=st, stop=sp)
                    nc.tensor.matmul(pyb[:], lhsT=lhs, rhs=hT[k][:, 512:CAP],
                                     start=st, stop=sp)
                nc.scalar.copy(yT[:, 0:512], pya[:])
                nc.scalar.copy(yT[:, 512:CAP], pyb[:])
                emit_preload()
                for g in range(G):
                    ps_tp = ps.tile([P, P], BF16, tag="pB", space="PSUM")
                    nc.tensor.transpose(out=ps_tp[:], in_=yT[:, g * P:(g + 1) * P],
                                        identity=idb[:])
                    nc.scalar.activation(out=ynat[g][:, mh * P:(mh + 1) * P],
                                         in_=ps_tp[:], func=AF.Copy,
                                         scale=cw_slots[:, g:g + 1])
            for g in range(G):
                nc.gpsimd.indirect_dma_start(
                    out=acc_d[:, :],
                    out_offset=bass.IndirectOffsetOnAxis(ap=idx_i[:, g:g + 1], axis=0),
                    in_=ynat[g][:], in_offset=None,
                    bounds_check=T - 1, oob_is_err=False)

            rw1p, rw3p, rw2p = rwpre[0:KH], rwpre[KH:2 * KH], rwpre[2 * KH:3 * KH]

            # D3: attention per head (all tiles base-partition 0)
            mk = [cpool.tile([P, TS], F32, name=f"mk{c}") for c in range(4)]
            for c in range(4):
                nc.sync.dma_start(out=mk[c][:], in_=maskT[c * P:(c + 1) * P, :])
            attnh = [res.tile([HD, TS], BF16, tag=(f"attnh{h}" if h < 2 else f"qrh{h - 2}"), name=f"attnh{h}") for h in range(NH)]
            for h in range(NH):
                kvh = h // 4
                pd = ps1.tile([P, TS], F32, tag="pd", space="PSUM")
                po = ps1.tile([HD, TS], F32, tag="po", space="PSUM")
                for c in range(4):
                    ps_s = ps.tile([P, TS], F32, tag="pC", space="PSUM")
                    nc.tensor.matmul(ps_s[:],
                                     lhsT=krh[kvh][:, c * P:(c + 1) * P],
                                     rhs=qrh[h][:], start=True, stop=True)
                    sm = sb.tile([P, TS], F32, tag="sm")
                    nc.vector.tensor_add(out=sm[:], in0=ps_s[:], in1=mk[c][:])
                    pT = sb.tile([P, TS], BF16, tag="pT", bufs=3)
                    nc.scalar.activation(out=pT[:], in_=sm[:], func=AF.Exp, scale=0.125)
                    nc.tensor.matmul(pd[:], lhsT=ones_b[:], rhs=pT[:],
                                     start=(c == 0), stop=(c == 3))
                    nc.tensor.matmul(po[:], lhsT=vnat[c][:, kvh * HD:(kvh + 1) * HD],
                                     rhs=pT[:], start=(c == 0), stop=(c == 3))
                rd = sb.tile([HD, TS], F32, tag="rd")
                nc.vector.reciprocal(rd[:], pd[0:HD, :])
                nc.vector.tensor_tensor(out=attnh[h][:], in0=po[:],
                                        in1=rd[:], op=OP.mult)

            # D4: output projection (contraction in 16 chunks of 64) + residual
            RAT = [res.tile([P, TS], F32, tag=f"xgT{m}", name=f"RAT{m}") for m in range(KH)]
            for m in range(KH):
                woa = sb.tile([HD, 8 * P], BF16, tag="wom", bufs=2, name="woa")
                nc.sync.dma_start(
                    out=woa[:],
                    in_=wo[m, :, 0:8 * P])
                wob = sb.tile([HD, 8 * P], BF16, tag="wom", bufs=2, name="wob")
                nc.sync.dma_start(
                    out=wob[:],
                    in_=wo[m, :, 8 * P:NH * P])
                op_ps = ps.tile([P, TS], F32, tag="pB", space="PSUM")
                for k in range(NH):
                    wsrc = woa if k < 8 else wob
                    nc.tensor.matmul(op_ps[:], lhsT=wsrc[:, (k % 8) * P:(k % 8 + 1) * P],
                                     rhs=attnh[k][:], start=(k == 0), stop=(k == NH - 1))
                xres = sb.tile([P, TS], F32, tag="xres", name="xres")
                nc.sync.dma_start(out=xres[:], in_=xT_kv[m * P:(m + 1) * P, TS:KV])
                nc.vector.tensor_add(out=RAT[m][:], in0=op_ps[:], in1=xres[:])

            # D5: residual MLP
            ps_rm = ps.tile([P, TS], F32, tag="pA", space="PSUM")
            for m in range(KH):
                sqm = sb.tile([P, TS], BF16, tag="sqm")
                nc.scalar.activation(out=sqm[:], in_=RAT[m][:], func=AF.Square)
                nc.tensor.matmul(ps_rm[:], lhsT=ones_b[:], rhs=sqm[:],
                                 start=(m == 0), stop=(m == KH - 1))
            srm = sb.tile([P, TS], F32, tag="srm")
            nc.scalar.activation(out=srm[:], in_=ps_rm[:], func=AF.Sqrt,
                                 scale=1.0 / H, bias=epsb[:])
            rrm = sb.tile([P, TS], F32, tag="rrm", bufs=1)
            nc.vector.reciprocal(rrm[:], srm[:])
            xmT = [res.tile([P, TS], BF16, tag=f"hT{16 + m}" if m < 6 else f"ynat{m - 6}", name=f"xmT{m}") for m in range(KH)]
            for m in range(KH):
                nc.vector.tensor_mul(out=xmT[m][:], in0=RAT[m][:], in1=rrm[:])
            hm = [res.tile([P, TS], BF16, tag=f"hT{8 + m}", name=f"hm{m}") for m in range(KH)]
            for m in range(KH):
                p1 = ps.tile([P, TS], F32, tag="pB", space="PSUM")
                for k in range(KH):
                    nc.tensor.matmul(p1[:], lhsT=rw1p[m][:, k * P:(k + 1) * P],
                                     rhs=xmT[k][:], start=(k == 0), stop=(k == KH - 1))
                p3 = ps.tile([P, TS], F32, tag="pC", space="PSUM")
                for k in range(KH):
                    nc.tensor.matmul(p3[:], lhsT=rw3p[m][:, k * P:(k + 1) * P],
                                     rhs=xmT[k][:], start=(k == 0), stop=(k == KH - 1))
                t1 = sb.tile([P, TS], BF16, tag="t1d")
                nc.scalar.activation(out=t1[:], in_=p1[:], func=AF.Sigmoid)
                tb = sb.tile([P, TS], BF16, tag="tbd")
                nc.vector.tensor_tensor(out=tb[:], in0=t1[:], in1=p1[:], op=OP.mult)
                nc.vector.tensor_tensor(out=hm[m][:], in0=tb[:], in1=p3[:], op=OP.mult)

            # D6a: rw2 + residual accumulated in place into RAT (pre-collective)
            for m in range(KH):
                p2 = ps.tile([P, TS], F32, tag="pB", space="PSUM")
                for k in range(KH):
                    nc.tensor.matmul(p2[:], lhsT=rw2p[m][:, k * P:(k + 1) * P],
                                     rhs=hm[k][:], start=(k == 0), stop=(k == KH - 1))
                nc.vector.tensor_add(out=RAT[m][:], in0=p2[:], in1=RAT[m][:])
            # ---------------- M8: ReduceScatter ----------------
            nc.gpsimd.collective_compute(
                "ReduceScatter", OP.add, replica_groups=[list(range(NCORES))],
                ins=[acc_d.ap().opt()], outs=[rs_d.ap().opt()])

            # D6b: MoE slice transpose fused into final sum -> output
            ots = [sb.tile([P, TS], F32, tag=f"hT{m}", name=f"ot{m}", bufs=1)
                   for m in range(KH)]
            for pt in range(2):
                rsb = sb.tile([P, H], BF16, tag="rsb")
                nc.sync.dma_start(out=rsb[:], in_=rs_d[pt * P:(pt + 1) * P, :])
                for k in range(KH):
                    ps_tp = ps.tile([P, P], BF16, tag="pB", space="PSUM")
                    nc.tensor.transpose(out=ps_tp[:], in_=rsb[:, k * P:(k + 1) * P],
                                        identity=idb[:])
                    nc.vector.tensor_add(out=ots[k][:, pt * P:(pt + 1) * P],
                                         in0=ps_tp[:],
                                         in1=RAT[k][:, pt * P:(pt + 1) * P])
            for m in range(KH):
                nc.sync.dma_start(out=out[m * P:(m + 1) * P, :], in_=ots[m][:])

    nc.finalize()
    _BUILD_CACHE["nc"] = nc
    return nc


def _host_prep(inputs):
    f32 = np.float32
    x = np.asarray(inputs["hidden_states"], f32).reshape(T, H)
    ln1 = np.asarray(inputs["ln1_w"], f32)
    res_ln = np.asarray(inputs["res_ln_w"], f32)
    post_ln = np.asarray(inputs["post_ln_w"], f32)

    import ml_dtypes
    bf16 = ml_dtypes.bfloat16

    def b(a):
        return np.ascontiguousarray(np.asarray(a, f32)).astype(bf16)

    def mmaj(w, pp, mm):
        # [K, M] -> [M//mm, pp, (K//pp)*mm] with w[k, m] at [m//mm, k%pp, (k//pp)*mm + m%mm]
        K, M = w.shape
        return np.ascontiguousarray(
            w.reshape(K // pp, pp, M // mm, mm).transpose(2, 1, 0, 3).reshape(M // mm, pp, (K // pp) * mm))

    wq = mmaj(b(ln1[:, None] * np.asarray(inputs["q_w"], f32)), 128, 64)
    wk = mmaj(b(ln1[:, None] * np.asarray(inputs["k_w"], f32)), 128, 64)
    wv = mmaj(b(ln1[:, None] * np.asarray(inputs["v_w"], f32)), 128, 128)
    wo = mmaj(b(inputs["o_w"]), 64, 128)
    rw1 = mmaj(b(res_ln[:, None] * np.asarray(inputs["rw1"], f32)), 128, 128)
    rw3 = mmaj(b(res_ln[:, None] * np.asarray(inputs["rw3"], f32)), 128, 128)
    rw2 = mmaj(b(inputs["rw2"]), 128, 128)
    gate = np.ascontiguousarray(post_ln[:, None] * np.asarray(inputs["gate_w"], f32))
    gatep = np.ascontiguousarray(gate.reshape(8, 128, 8).transpose(1, 0, 2).reshape(128, 64))
    xT = np.ascontiguousarray(x.T)                       # [H, T]

    e_w1 = np.asarray(inputs["e_w1"], f32)
    e_w3 = np.asarray(inputs["e_w3"], f32)
    e_w2 = np.asarray(inputs["e_w2"], f32)

    # RoPE tables: cos64[d, pos] with d in [0,64), duplicated inv-freq halves
    pos = np.arange(S, dtype=f32)
    inv = 1.0 / (THETA ** (np.arange(0, HD, 2, dtype=f32) / HD))   # [32]
    ang = inv[:, None] * pos[None, :]                               # [32, S]
    cos64 = np.concatenate([np.cos(ang)] * 2, 0)                    # [64, S]
    sin64 = np.concatenate([np.sin(ang)] * 2, 0)

    in_maps = []
    for core in range(NCORES):
        bi, c = divmod(core, 4)
        lo = bi * S + c * TS
        # kv window: previous chunk + own chunk (zeros for c == 0)
        xkv = np.zeros((H, KV), f32)
        if c > 0:
            xkv[:, :TS] = xT[:, lo - TS:lo]
        xkv[:, TS:] = xT[:, lo:lo + TS]
        # mask: valid iff ql < kl <= ql + TS (and kl >= TS when c == 0)
        ql = np.arange(TS)[None, :]
        kl = np.arange(KV)[:, None]
        valid = (kl > ql) & (kl <= ql + TS)
        if c == 0:
            valid &= kl >= TS
        maskT = np.where(valid, 0.0, NEG).astype(f32)
        # RoPE positions (within-sequence)
        pq = c * TS + np.arange(TS)
        pk = np.clip((c - 1) * TS + np.arange(KV), 0, S - 1)
        cq = np.tile(cos64[:, pq], (2, 1)).astype(f32)
        sqv = np.tile(sin64[:, pq], (2, 1)).astype(f32)
        ckv = np.tile(cos64[:, pk], (2, 1)).astype(f32)
        skv = np.tile(sin64[:, pk], (2, 1)).astype(f32)
        oh = np.zeros((P, E), f32)
        oh[:, core] = 1.0
        in_maps.append(dict(
            xT_kv=xkv, xnat=x.astype(bf16), xT=xT, gatep=gatep, onehot=oh,
            cos_q=cq, sin_q=sqv, cos_k=ckv, sin_k=skv, maskT=maskT,
            wq=wq, wk=wk, wv=wv, wo=wo, rw1=rw1, rw3=rw3, rw2=rw2,
            ew1=mmaj(b(post_ln[:, None] * e_w1[core]), 128, 128),
            ew3=mmaj(b(post_ln[:, None] * e_w3[core]), 128, 128),
            ew2=mmaj(b(e_w2[core]), 128, 128),
        ))
    return in_maps


def kernel(**inputs) -> np.ndarray:
    nc = _build()
    in_maps = _host_prep(inputs)
    res = run_bass_kernel_spmd(nc, in_maps, core_ids=list(range(NCORES)))
    outs = [np.asarray(res.results[i]["out"], np.float32).T for i in range(NCORES)]
    full = np.concatenate(outs, 0)          # [T, H] in core order == token order
    return full.reshape(B, S, H)
